# revision 22
# baseline (speedup 1.0000x reference)
"""DeltaAttention Trainium2 kernel — 8-core SPMD via bass/Tile.

Math (per reference): 4 DeltaResidualBlocks (d_v=1) wrapped around MHA.
Because each delta block consumes its v_in only through the scalar
projection v_in @ dWv[i], the Wq/Wk/Wv/Wo matmuls collapse into single
extra columns of the dWk matmuls (precomputed on host), and attn@v
collapses to 2 output columns per head:
    n_h[q] = E_h[q,:] @ u_h,  r_h[q] = E_h[q,:] @ 1,  u_h = v_h @ w_h
    v3[q]  = sum_h n_h/r_h + const,   w = Wo @ dWv[3]
Sharding: 512 query tokens per core; k^T and u are AllGathered within
each 4-core batch group.  LayerNorm statistics are computed from
precomputed moments of x and k3_raw (y = x + s3*k3_raw).
"""

import os
from contextlib import ExitStack

import numpy as np
import ml_dtypes

import concourse.bass as bass
import concourse.mybir as mybir
import concourse.tile as tile
from concourse.bass_utils import run_bass_kernel_spmd
from concourse.masks import make_identity

dt = mybir.dt
AF = mybir.ActivationFunctionType
ALU = mybir.AluOpType
ts = bass.ts

N_CORES = 8
B, S, D, H = 2, 2048, 1024, 16
HD = D // H
TOK = (B * S) // N_CORES          # 512 query tokens per core
M4 = TOK // 128                   # 4 token chunks
K8 = D // 128                     # 8 feature chunks
NKC = S // 128                    # 16 key chunks per batch
EPS = 1e-8
LN_EPS = 1e-5

# extras matmul columns: [dbw0,vw0, dbw1,vw1, dbw2,vw2, Wu(16), Bu(16), dbw3]
W_EX = 39
EX_DBW = [0, 2, 4, 38]
EX_VW = [1, 3, 5]
EX_A = 6      # 6..22  = x @ Wu
EX_B = 22     # 22..38 = x @ dWk2 @ Wu

# dtype of q^T / k^T used by the scores matmul (fp8 halves the AllGather)
SC_DT = dt.float8e4 if os.environ.get("DELTA_SCORES_FP8", "1") == "1" else dt.bfloat16

LAST_RESULTS = None
_CACHE = {}


def _split_multi_waits(nc, max_waits=1):
    """walrus (CoreV3) only encodes one sync wait per instruction; Tile's
    final drain can carry several. Hoist extras onto preceding NoOps."""
    n_fixed = 0
    for f in nc.m.functions:
        for blk in f.blocks:
            new_insts = []
            for inst in blk.instructions:
                si = inst.sync_info
                waits = list(si.on_wait) if (si and si.on_wait) else []
                if len(waits) > max_waits:
                    head, tail = waits[:-max_waits], waits[-max_waits:]
                    for j, w in enumerate(head):
                        nop = mybir.InstNoOp(
                            name=f"{inst.name}_waitsplit_{j}",
                            engine=inst.engine,
                            ins=[],
                            outs=[],
                            sync_info=mybir.SyncInfo(on_wait=[w], on_update=[]),
                        )
                        nc.register_instruction(nop)
                        new_insts.append(nop)
                        n_fixed += 1
                    si.on_wait = tail
                new_insts.append(inst)
            blk.instructions[:] = new_insts
    return n_fixed


def _build_program():
    nc = bass.Bass(num_devices=N_CORES)

    x_t = nc.dram_tensor("x", [TOK, D], dt.float32, kind="ExternalInput")
    aug_t = [
        nc.dram_tensor(f"aug{i}", [D, D], dt.bfloat16, kind="ExternalInput")
        for i in range(4)
    ]
    ex_t = nc.dram_tensor("ex", [D, W_EX], dt.bfloat16, kind="ExternalInput")
    cvec_t = nc.dram_tensor("cvec", [128, 16], dt.float32, kind="ExternalInput")
    lng_t = nc.dram_tensor("lng", [128, D], dt.float32, kind="ExternalInput")
    lnb_t = nc.dram_tensor("lnb", [128, D], dt.float32, kind="ExternalInput")
    y_t = nc.dram_tensor("y", [TOK, D], dt.float32, kind="ExternalOutput")

    RG = [[0, 1, 2, 3], [4, 5, 6, 7]]

    with tile.TileContext(nc) as tc, ExitStack() as stack:
        const = stack.enter_context(tc.tile_pool(name="const", bufs=1))
        dram = stack.enter_context(tc.tile_pool(name="dram", bufs=1, space="DRAM"))
        big = stack.enter_context(tc.tile_pool(name="big", bufs=1))

        agk_in = dram.tile([D, TOK], SC_DT, tag="agk_in")
        agk_out = dram.tile([4 * D, TOK], SC_DT, tag="agk_out")
        agu_in = dram.tile([TOK, H], dt.bfloat16, tag="agu_in")
        agu_out = dram.tile([4 * TOK, H], dt.bfloat16, tag="agu_out")

        ident_bf = const.tile([128, 128], dt.bfloat16, tag="ident_bf")
        make_identity(nc, ident_bf[:])
        ident_f32 = const.tile([128, 128], dt.float32, tag="ident_f32")
        make_identity(nc, ident_f32[:])
        cvec = const.tile([128, 16], dt.float32, tag="cvec")
        nc.sync.dma_start(cvec[:], cvec_t[:])
        lng = const.tile([128, D], dt.float32, tag="lng")
        lnb = const.tile([128, D], dt.float32, tag="lnb")
        nc.sync.dma_start(lng[:], lng_t[:])
        nc.sync.dma_start(lnb[:], lnb_t[:])

        # persistent data tiles
        x32 = [big.tile([128, D], dt.float32, tag=f"x32_{m}", name=f"x32_{m}") for m in range(M4)]
        xbf = [big.tile([128, D], dt.bfloat16, tag=f"xbf_{m}", name=f"xbf_{m}") for m in range(M4)]
        xT = [big.tile([128, TOK], dt.bfloat16, tag=f"xT_{k}", name=f"xT_{k}") for k in range(K8)]
        qT = [big.tile([128, TOK], SC_DT, tag=f"qT_{k}", name=f"qT_{k}") for k in range(K8)]
        k3raw = [big.tile([128, D], dt.bfloat16, tag=f"k3_{m}", name=f"k3_{m}") for m in range(M4)]
        a3s = [big.tile([128, 1], dt.float32, tag=f"a3_{m}", name=f"a3_{m}") for m in range(M4)]
        b3s = [big.tile([128, 1], dt.float32, tag=f"b3_{m}", name=f"b3_{m}") for m in range(M4)]
        u_bf = [big.tile([128, H], dt.bfloat16, tag=f"u_{m}", name=f"u_{m}") for m in range(M4)]
        exsb = [big.tile([128, W_EX], dt.float32, tag=f"ex_{m}", name=f"ex_{m}") for m in range(M4)]
        v3acc = [big.tile([128, 1], dt.float32, tag=f"v3a_{m}", name=f"v3a_{m}") for m in range(M4)]
        mxs = [big.tile([128, 1], dt.float32, tag=f"mx_{m}", name=f"mx_{m}") for m in range(M4)]
        xxs = [big.tile([128, 1], dt.float32, tag=f"xx_{m}", name=f"xx_{m}") for m in range(M4)]
        mks3 = [big.tile([128, 1], dt.float32, tag=f"mk3_{m}", name=f"mk3_{m}") for m in range(M4)]
        kks3 = [big.tile([128, 1], dt.float32, tag=f"kk3_{m}", name=f"kk3_{m}") for m in range(M4)]
        xks3 = [big.tile([128, 1], dt.float32, tag=f"xk3_{m}", name=f"xk3_{m}") for m in range(M4)]
        nrw = big.tile([2, 2 * TOK], dt.float32, tag="nrw")
        aug3t = [
            big.tile([128, D], dt.bfloat16, tag=f"aug3_{k}", name=f"aug3w_{k}")
            for k in range(K8)
        ]

        for m in range(M4):
            nc.sync.dma_start(x32[m][:], x_t[ts(m, 128), :])
            nc.scalar.copy(xbf[m][:], x32[m][:])
            nc.vector.memset(v3acc[m][:], 0.0)
            nc.vector.tensor_reduce(mxs[m][:], x32[m][:], axis=mybir.AxisListType.X, op=ALU.add)
        for k in range(K8):
            nc.sync.dma_start(aug3t[k][:], aug_t[3][ts(k, 128), :])

        with (
            tc.tile_pool(name="wpool", bufs=16) as wpool,
            tc.tile_pool(name="qkpool", bufs=4) as qkpool,
            tc.tile_pool(name="scpool", bufs=24) as scpool,
            tc.tile_pool(name="scr", bufs=2) as scrpool,
            tc.tile_pool(name="ktloc", bufs=8) as ktlpool,
            tc.tile_pool(name="pp_proj", bufs=2, space="PSUM") as pp_proj,
            tc.tile_pool(name="pp_ex", bufs=2, space="PSUM") as pp_ex,
            tc.tile_pool(name="pp_t", bufs=2, space="PSUM") as pp_t,
        ):
            # x^T via PE transpose (bf16)
            for k in range(K8):
                pst = pp_t.tile([128, TOK], dt.bfloat16, tag="pst")
                for m in range(M4):
                    nc.tensor.transpose(
                        pst[:, ts(m, 128)], xbf[m][:, ts(k, 128)], ident_bf[:]
                    )
                nc.vector.tensor_copy(xT[k][:], pst[:])

            # extras matmul: all betas / v-scalars / u components at once
            ext = [wpool.tile([128, W_EX], dt.bfloat16, tag="ext", name=f"ext_{k}") for k in range(K8)]
            for k in range(K8):
                nc.sync.dma_start(ext[k][:], ex_t[ts(k, 128), :])
            for m in range(M4):
                pse = pp_ex.tile([128, W_EX], dt.float32, tag="pse")
                for k in range(K8):
                    nc.tensor.matmul(
                        pse[:], xT[k][:, ts(m, 128)], ext[k][:],
                        start=(k == 0), stop=(k == K8 - 1),
                    )
                nc.vector.tensor_copy(exsb[m][:], pse[:])

            qk_out = {}

            def scalar_chain(i, m, ps_beta_src, kx, rnorm):
                """beta, rk, rr from per-chunk scalars. Returns (rk, rr)."""
                ez = scpool.tile([128, 1], dt.float32, tag="sc", name=f"ez_{i}_{m}")
                nc.scalar.activation(
                    ez[:], ps_beta_src, AF.Exp, scale=-1.0, bias=cvec[:, i:i + 1]
                )
                ez1 = scpool.tile([128, 1], dt.float32, tag="sc", name=f"ez1_{i}_{m}")
                nc.vector.tensor_scalar_add(ez1[:], ez[:], 1.0)
                rsig = scpool.tile([128, 1], dt.float32, tag="sc", name=f"rs_{i}_{m}")
                nc.vector.reciprocal(rsig[:], ez1[:])
                rk = scpool.tile([128, 1], dt.float32, tag="sc", name=f"rk_{i}_{m}")
                nc.vector.tensor_scalar_mul(rk[:], kx[:], rnorm[:])
                rr = scpool.tile([128, 1], dt.float32, tag="sc", name=f"rr_{i}_{m}")
                nc.vector.tensor_scalar(rr[:], rsig[:], rnorm[:], 2.0, ALU.mult, ALU.mult)
                return rk, rr

            def rnorm_chain(i, m, ss):
                lnv = scpool.tile([128, 1], dt.float32, tag="sc", name=f"lnv_{i}_{m}")
                nc.scalar.activation(lnv[:], ss[:], AF.Ln)
                nrm = scpool.tile([128, 1], dt.float32, tag="sc", name=f"nrm_{i}_{m}")
                nc.scalar.activation(nrm[:], lnv[:], AF.Exp, scale=0.5)
                nrme = scpool.tile([128, 1], dt.float32, tag="sc", name=f"nrme_{i}_{m}")
                nc.vector.tensor_scalar_add(nrme[:], nrm[:], EPS)
                rnorm = scpool.tile([128, 1], dt.float32, tag="sc", name=f"rn_{i}_{m}")
                nc.vector.reciprocal(rnorm[:], nrme[:])
                return rnorm

            def delta_block(i):
                """dWk matmul + delta elementwise for aug i on all 4 chunks."""
                augt = [
                    wpool.tile([128, D], dt.bfloat16, tag="aug", name=f"aug_{i}_{k}")
                    for k in range(K8)
                ]
                for k in range(K8):
                    nc.sync.dma_start(augt[k][:], aug_t[i][ts(k, 128), :])
                outs = []
                for m in range(M4):
                    ps = pp_proj.tile([128, D], dt.float32, tag="ps_proj")
                    for k in range(K8):
                        for s0 in (0, 512):
                            nc.tensor.matmul(
                                ps[:, s0:s0 + 512], xT[k][:, ts(m, 128)],
                                augt[k][:, s0:s0 + 512],
                                start=(k == 0), stop=(k == K8 - 1),
                            )
                    ex = exsb[m]
                    scr = scrpool.tile([128, D], dt.bfloat16, tag="scr", name=f"scr_{i}_{m}")
                    ss = scpool.tile([128, 1], dt.float32, tag="sc", name=f"ss_{i}_{m}")
                    nc.scalar.activation(scr[:], ps[:], AF.Square, accum_out=ss[:])
                    kx = scpool.tile([128, 1], dt.float32, tag="sc", name=f"kx_{i}_{m}")
                    scr2 = scrpool.tile([128, D], dt.bfloat16, tag="scr", name=f"scr2_{i}_{m}")
                    nc.vector.scalar_tensor_tensor(
                        scr2[:], ps[:], 1.0, x32[m][:], ALU.mult, ALU.mult,
                        accum_out=kx[:],
                    )
                    rnorm = rnorm_chain(i, m, ss)
                    rk, rr = scalar_chain(i, m, ex[:, EX_DBW[i]:EX_DBW[i] + 1], kx, rnorm)
                    v = scpool.tile([128, 1], dt.float32, tag="sc", name=f"v_{i}_{m}")
                    nc.vector.tensor_scalar_add(
                        v[:], ex[:, EX_VW[i]:EX_VW[i] + 1], cvec[:, 4 + i:5 + i]
                    )
                    dv = scpool.tile([128, 1], dt.float32, tag="sc", name=f"dv_{i}_{m}")
                    nc.vector.tensor_tensor(dv[:], v[:], rk[:], ALU.subtract)
                    s = scpool.tile([128, 1], dt.float32, tag="sc", name=f"s_{i}_{m}")
                    nc.vector.tensor_tensor(s[:], dv[:], rr[:], ALU.mult)
                    if i in (0, 1):
                        o = qkpool.tile([128, D], dt.bfloat16, tag="qk", name=f"qk_{i}_{m}")
                        nc.vector.scalar_tensor_tensor(
                            o[:], ps[:], s[:], x32[m][:], ALU.mult, ALU.add
                        )
                        outs.append(o)
                    else:
                        # i == 2: u = A + s*B  (A/B live in the extras tile)
                        nc.vector.scalar_tensor_tensor(
                            u_bf[m][:], ex[:, EX_B:EX_B + H], s[:], ex[:, EX_A:EX_A + H],
                            ALU.mult, ALU.add,
                        )
                qk_out[i] = outs

            def delta3_chunk(m):
                """dWk3 matmul; elementwise on DVE from SBUF copy; LN moments."""
                psd = pp_proj.tile([128, D], dt.float32, tag="ps_proj")
                for k in range(K8):
                    for s0 in (0, 512):
                        nc.tensor.matmul(
                            psd[:, s0:s0 + 512], xT[k][:, ts(m, 128)],
                            aug3t[k][:, s0:s0 + 512],
                            start=(k == 0), stop=(k == K8 - 1),
                        )
                mka = scpool.tile([128, 1], dt.float32, tag="sc", name=f"mka_{m}")
                mkb = scpool.tile([128, 1], dt.float32, tag="sc", name=f"mkb_{m}")
                nc.vector.tensor_scalar(
                    k3raw[m][:, 0:512], psd[:, 0:512], 1.0, 0.0, ALU.mult,
                    ALU.add, accum_out=mka[:],
                )
                nc.vector.tensor_scalar(
                    k3raw[m][:, 512:1024], psd[:, 512:1024], 1.0, 0.0, ALU.mult,
                    ALU.add, accum_out=mkb[:],
                )
                nc.vector.tensor_tensor(mks3[m][:], mka[:], mkb[:], ALU.add)
                scr = scrpool.tile([128, D], dt.bfloat16, tag="scr", name=f"sc3r_{m}")
                nc.vector.scalar_tensor_tensor(
                    scr[:], k3raw[m][:], 1.0, k3raw[m][:], ALU.mult, ALU.mult,
                    accum_out=kks3[m][:],
                )
                scr2 = scrpool.tile([128, D], dt.bfloat16, tag="scr", name=f"sc3r2_{m}")
                nc.vector.scalar_tensor_tensor(
                    scr2[:], k3raw[m][:], 1.0, xbf[m][:], ALU.mult, ALU.mult,
                    accum_out=xks3[m][:],
                )
                rnorm = rnorm_chain(3, m, kks3[m])
                rk, rr = scalar_chain(3, m, exsb[m][:, EX_DBW[3]:EX_DBW[3] + 1], xks3[m], rnorm)
                nc.vector.tensor_copy(a3s[m][:], rr[:])
                nc.vector.tensor_tensor(b3s[m][:], rr[:], rk[:], ALU.mult)

            def transpose_to(src_tiles, dst_tiles):
                for k in range(K8):
                    pst = pp_t.tile([128, TOK], dt.bfloat16, tag="pst")
                    for m in range(M4):
                        nc.tensor.transpose(
                            pst[:, ts(m, 128)], src_tiles[m][:, ts(k, 128)], ident_bf[:]
                        )
                    nc.vector.tensor_copy(dst_tiles[k][:], pst[:])

            # ---- k path first so the AllGather starts early
            delta_block(1)
            ktloc = [ktlpool.tile([128, TOK], SC_DT, tag="ktloc", name=f"ktloc_{k}") for k in range(K8)]
            transpose_to(qk_out[1], ktloc)
            for k in range(K8):
                nc.sync.dma_start(agk_in[ts(k, 128), :], ktloc[k][:])
            nc.gpsimd.collective_compute(
                "AllGather", ALU.bypass, ins=[agk_in[:]], outs=[agk_out[:]],
                replica_groups=RG,
            )
            delta_block(2)
            for m in range(M4):
                nc.sync.dma_start(agu_in[ts(m, 128), :], u_bf[m][:])
            nc.gpsimd.collective_compute(
                "AllGather", ALU.bypass, ins=[agu_in[:]], outs=[agu_out[:]],
                replica_groups=RG,
            )
            delta_block(0)
            transpose_to(qk_out[0], qT)
            for m in range(M4):
                delta3_chunk(m)
                xsq = scrpool.tile([128, D], dt.bfloat16, tag="scr", name=f"xsq_{m}")
                nc.scalar.activation(xsq[:], x32[m][:], AF.Square, accum_out=xxs[m][:])

        # ---------------- attention ----------------
        with (
            tc.tile_pool(name="attn_sb", bufs=1) as attn_sb,
            tc.tile_pool(name="epool", bufs=4) as epool,
            tc.tile_pool(name="fin", bufs=2) as fin,
            tc.tile_pool(name="pp_sc", bufs=3, space="PSUM") as pp_sc,
            tc.tile_pool(name="pp_nr", bufs=2, space="PSUM") as pp_nr,
        ):
            kT = [attn_sb.tile([128, S], SC_DT, tag=f"kT_{k}", name=f"kTsb_{k}") for k in range(K8)]
            for k in range(K8):
                src = agk_out[:].rearrange("(c d) t -> d c t", c=4)[ts(k, 128), :, :]
                dst = kT[k][:].rearrange("p (c t) -> p c t", c=4)
                nc.gpsimd.dma_start(dst, src)
            uext = attn_sb.tile([128, NKC, H, 2], dt.bfloat16, tag="uext")
            nc.vector.memset(uext[:], 1.0)
            u_all = attn_sb.tile([128, NKC, H], dt.bfloat16, tag="u_all")
            nc.gpsimd.dma_start(
                u_all[:], agu_out[:].rearrange("(kc p) h -> p kc h", p=128)
            )
            nc.vector.tensor_copy(uext[:, :, :, 0], u_all[:])

            SCALE = float(HD) ** -0.5

            for hp in range(K8):         # 8 head pairs; pair hp = heads 2hp, 2hp+1
                nr_ps = pp_nr.tile([128, TOK], dt.float32, tag="nr")
                hA, hB = 2 * hp, 2 * hp + 1
                for kc in range(NKC):
                    ps2 = pp_sc.tile([128, 2, TOK], dt.float32, tag="sc2")
                    nc.tensor.matmul(
                        ps2[:, 0, :], kT[hp][0:64, ts(kc, 128)], qT[hp][0:64, :],
                        start=True, stop=True, tile_position=(0, 0),
                    )
                    nc.tensor.matmul(
                        ps2[:, 1, :], kT[hp][64:128, ts(kc, 128)], qT[hp][64:128, :],
                        start=True, stop=True, tile_position=(64, 0),
                    )
                    E = epool.tile([128, 2, TOK], dt.bfloat16, tag="E")
                    nc.scalar.activation(E[:], ps2[:], AF.Exp, scale=SCALE)
                    nc.tensor.matmul(
                        nr_ps[0:2, :], uext[:, kc, hA, :], E[:, 0, :],
                        start=(kc == 0), stop=(kc == NKC - 1),
                        tile_position=(0, 0),
                    )
                    nc.tensor.matmul(
                        nr_ps[32:34, :], uext[:, kc, hB, :], E[:, 1, :],
                        start=(kc == 0), stop=(kc == NKC - 1),
                        tile_position=(0, 32),
                    )
                # stage the pair's n/r rows and fold into v3acc (overlaps attention)
                for j in range(2):
                    nc.vector.tensor_copy(
                        nrw[0:2, j * TOK:(j + 1) * TOK], nr_ps[32 * j:32 * j + 2, :]
                    )
                for m in range(M4):
                    psT = pp_sc.tile([128, 2, TOK], dt.float32, tag="sc2")
                    for j in range(2):
                        nc.tensor.transpose(
                            psT[:, 0, 2 * j:2 * j + 2],
                            nrw[0:2, j * TOK + 128 * m: j * TOK + 128 * (m + 1)],
                            ident_f32[0:2, 0:2],
                        )
                    nrT = fin.tile([128, 4], dt.float32, tag="nrT", name=f"nrT_{hp}_{m}")
                    nc.vector.tensor_copy(nrT[:], psT[:, 0, 0:4])
                    rec = fin.tile([128, 2], dt.float32, tag="rec", name=f"rec_{hp}_{m}")
                    nc.vector.reciprocal(rec[:], nrT[:, 1:4:2])
                    prod = fin.tile([128, 2], dt.float32, tag="prod", name=f"pr_{hp}_{m}")
                    nc.vector.tensor_tensor(prod[:], nrT[:, 0:4:2], rec[:], ALU.mult)
                    pv = fin.tile([128, 1], dt.float32, tag="pv", name=f"pv_{hp}_{m}")
                    nc.vector.tensor_reduce(pv[:], prod[:], axis=mybir.AxisListType.X, op=ALU.add)
                    nc.vector.tensor_tensor(v3acc[m][:], v3acc[m][:], pv[:], ALU.add)

            # ---- final delta + layernorm (stats from precomputed moments)
            for m in range(M4):
                v3 = fin.tile([128, 1], dt.float32, tag="v3", name=f"v3_{m}")
                nc.vector.tensor_scalar_add(v3[:], v3acc[m][:], cvec[:, 7:8])
                s3 = fin.tile([128, 1], dt.float32, tag="s3", name=f"s3_{m}")
                nc.vector.tensor_scalar_mul(s3[:], v3[:], a3s[m][:])
                nc.vector.tensor_tensor(s3[:], s3[:], b3s[m][:], ALU.subtract)
                # mu = (sum_x + s3*sum_k)/D
                mu = fin.tile([128, 1], dt.float32, tag="mu", name=f"mu_{m}")
                nc.vector.tensor_scalar_mul(mu[:], s3[:], mks3[m][:])
                nc.vector.tensor_tensor(mu[:], mu[:], mxs[m][:], ALU.add)
                nc.vector.tensor_scalar_mul(mu[:], mu[:], 1.0 / D)
                # E[y^2] = (xx + 2 s3 xk + s3^2 kk)/D ; var = E[y^2] - mu^2
                t1 = fin.tile([128, 1], dt.float32, tag="t1", name=f"t1_{m}")
                nc.vector.tensor_scalar_mul(t1[:], s3[:], kks3[m][:])
                t2 = fin.tile([128, 1], dt.float32, tag="t2", name=f"t2_{m}")
                nc.vector.tensor_scalar(t2[:], xks3[m][:], 2.0, None, ALU.mult)
                nc.vector.tensor_tensor(t2[:], t2[:], t1[:], ALU.add)
                nc.vector.tensor_scalar_mul(t2[:], t2[:], s3[:])
                nc.vector.tensor_tensor(t2[:], t2[:], xxs[m][:], ALU.add)
                var = fin.tile([128, 1], dt.float32, tag="var", name=f"var_{m}")
                nc.vector.tensor_scalar_mul(var[:], t2[:], 1.0 / D)
                mu2 = fin.tile([128, 1], dt.float32, tag="mu2", name=f"mu2_{m}")
                nc.vector.tensor_tensor(mu2[:], mu[:], mu[:], ALU.mult)
                nc.vector.tensor_tensor(var[:], var[:], mu2[:], ALU.subtract)
                nc.vector.tensor_scalar_add(var[:], var[:], LN_EPS)
                lnv2 = fin.tile([128, 1], dt.float32, tag="lnv2", name=f"lnv2_{m}")
                nc.scalar.activation(lnv2[:], var[:], AF.Ln)
                rstd = fin.tile([128, 1], dt.float32, tag="rstd", name=f"rstd_{m}")
                nc.scalar.activation(rstd[:], lnv2[:], AF.Exp, scale=-0.5)
                s3r = fin.tile([128, 1], dt.float32, tag="s3r", name=f"s3r_{m}")
                nc.vector.tensor_scalar_mul(s3r[:], s3[:], rstd[:])
                # yn = (x - mu)*rstd + k3raw*(s3*rstd) ; out = yn*g + b
                yn = fin.tile([128, D], dt.float32, tag="yn", name=f"yn_{m}")
                nc.vector.tensor_scalar(yn[:], x32[m][:], mu[:], rstd[:], ALU.subtract, ALU.mult)
                nc.vector.scalar_tensor_tensor(
                    yn[:], k3raw[m][:], s3r[:], yn[:], ALU.mult, ALU.add
                )
                yg = fin.tile([128, D], dt.float32, tag="yg", name=f"yg_{m}")
                nc.vector.tensor_tensor(yg[:], yn[:], lng[:], ALU.mult)
                nc.vector.tensor_tensor(yg[:], yg[:], lnb[:], ALU.add)
                nc.sync.dma_start(y_t[ts(m, 128), :], yg[:])

    _split_multi_waits(nc)
    nc.finalize()
    return nc


def _host_prep(inputs):
    """Precompute augmented weights and constants; returns per-core in_maps."""
    f32 = np.float32
    x = np.asarray(inputs["x"], f32)
    Wq, bq = np.asarray(inputs["Wq"], f32), np.asarray(inputs["bq"], f32)
    Wk, bk = np.asarray(inputs["Wk"], f32), np.asarray(inputs["bk"], f32)
    Wv, bv = np.asarray(inputs["Wv"], f32), np.asarray(inputs["bv"], f32)
    Wo, bo = np.asarray(inputs["Wo"], f32), np.asarray(inputs["bo"], f32)
    dWk, dbw = np.asarray(inputs["dWk"], f32), np.asarray(inputs["dbw"], f32)
    dbb, dWv = np.asarray(inputs["dbb"], f32), np.asarray(inputs["dWv"], f32)
    dbv = np.asarray(inputs["dbv"], f32)
    ln_g, ln_b = np.asarray(inputs["ln_g"], f32), np.asarray(inputs["ln_b"], f32)

    w = Wo @ dWv[3]                                   # (D,)
    Wu = np.zeros((D, H), f32)
    for h in range(H):
        Wu[h * HD:(h + 1) * HD, h] = w[h * HD:(h + 1) * HD]
    Bu = dWk[2] @ Wu                                  # (D, H)

    vw = [Wq @ dWv[0], Wk @ dWv[1], Wv @ dWv[2]]
    vc = [float(bq @ dWv[0] + dbv[0]), float(bk @ dWv[1] + dbv[1]),
          float(bv @ dWv[2] + dbv[2])]
    c3 = float(bo @ dWv[3] + dbv[3])

    bf = ml_dtypes.bfloat16
    augs = [np.ascontiguousarray(dWk[i]).astype(bf) for i in range(4)]
    ex = np.zeros((D, W_EX), f32)
    for i in range(4):
        ex[:, EX_DBW[i]] = dbw[i]
    for i in range(3):
        ex[:, EX_VW[i]] = vw[i]
    ex[:, EX_A:EX_A + H] = Wu
    ex[:, EX_B:EX_B + H] = Bu
    ex = ex.astype(bf)

    cvec = np.zeros((128, 16), f32)
    for i in range(4):
        cvec[:, i] = -dbb[i]
    for i in range(3):
        cvec[:, 4 + i] = vc[i]
    cvec[:, 7] = c3

    lng = np.broadcast_to(ln_g[None, :], (128, D)).copy()
    lnb = np.broadcast_to(ln_b[None, :], (128, D)).copy()

    xf = x.reshape(B * S, D)
    in_maps = []
    for c in range(N_CORES):
        m = {
            "x": np.ascontiguousarray(xf[c * TOK:(c + 1) * TOK]),
            "ex": ex, "cvec": cvec, "lng": lng, "lnb": lnb,
        }
        for i in range(4):
            m[f"aug{i}"] = augs[i]
        in_maps.append(m)
    return in_maps


def kernel(**inputs):
    global LAST_RESULTS
    if "nc" not in _CACHE:
        _CACHE["nc"] = _build_program()
    nc = _CACHE["nc"]
    in_maps = _host_prep(inputs)
    res = run_bass_kernel_spmd(nc, in_maps, core_ids=list(range(N_CORES)))
    LAST_RESULTS = res
    out = np.concatenate(
        [res.results[c]["y"] for c in range(N_CORES)], axis=0
    ).reshape(B, S, D)
    return out.astype(np.float32)


# revision 23
# speedup vs baseline: 1.0383x; 1.0383x over previous
"""DeltaAttention Trainium2 kernel — 8-core SPMD via bass/Tile.

Math (per reference): 4 DeltaResidualBlocks (d_v=1) wrapped around MHA.
Because each delta block consumes its v_in only through the scalar
projection v_in @ dWv[i], the Wq/Wk/Wv/Wo matmuls collapse into single
extra columns of the dWk matmuls (precomputed on host), and attn@v
collapses to 2 output columns per head:
    n_h[q] = E_h[q,:] @ u_h,  r_h[q] = E_h[q,:] @ 1,  u_h = v_h @ w_h
    v3[q]  = sum_h n_h/r_h + const,   w = Wo @ dWv[3]
Sharding: 512 query tokens per core; k^T and u are AllGathered within
each 4-core batch group.  LayerNorm statistics are computed from
precomputed moments of x and k3_raw (y = x + s3*k3_raw).
"""

import os
from contextlib import ExitStack

import numpy as np
import ml_dtypes

import concourse.bass as bass
import concourse.mybir as mybir
import concourse.tile as tile
from concourse.bass_utils import run_bass_kernel_spmd
from concourse.masks import make_identity

dt = mybir.dt
AF = mybir.ActivationFunctionType
ALU = mybir.AluOpType
ts = bass.ts

N_CORES = 8
B, S, D, H = 2, 2048, 1024, 16
HD = D // H
TOK = (B * S) // N_CORES          # 512 query tokens per core
M4 = TOK // 128                   # 4 token chunks
K8 = D // 128                     # 8 feature chunks
NKC = S // 128                    # 16 key chunks per batch
EPS = 1e-8
LN_EPS = 1e-5

# extras matmul columns: [dbw0,vw0, dbw1,vw1, dbw2,vw2, Wu(16), Bu(16), dbw3]
W_EX = 39
EX_DBW = [0, 2, 4, 38]
EX_VW = [1, 3, 5]
EX_A = 6      # 6..22  = x @ Wu
EX_B = 22     # 22..38 = x @ dWk2 @ Wu

# dtype of q^T / k^T used by the scores matmul (fp8 halves the AllGather)
SC_DT = dt.float8e4 if os.environ.get("DELTA_SCORES_FP8", "1") == "1" else dt.bfloat16

LAST_RESULTS = None
_CACHE = {}


def _split_multi_waits(nc, max_waits=1):
    """walrus (CoreV3) only encodes one sync wait per instruction; Tile's
    final drain can carry several. Hoist extras onto preceding NoOps."""
    n_fixed = 0
    for f in nc.m.functions:
        for blk in f.blocks:
            new_insts = []
            for inst in blk.instructions:
                si = inst.sync_info
                waits = list(si.on_wait) if (si and si.on_wait) else []
                if len(waits) > max_waits:
                    head, tail = waits[:-max_waits], waits[-max_waits:]
                    for j, w in enumerate(head):
                        nop = mybir.InstNoOp(
                            name=f"{inst.name}_waitsplit_{j}",
                            engine=inst.engine,
                            ins=[],
                            outs=[],
                            sync_info=mybir.SyncInfo(on_wait=[w], on_update=[]),
                        )
                        nc.register_instruction(nop)
                        new_insts.append(nop)
                        n_fixed += 1
                    si.on_wait = tail
                new_insts.append(inst)
            blk.instructions[:] = new_insts
    return n_fixed


def _build_program():
    nc = bass.Bass(num_devices=N_CORES)

    x_t = nc.dram_tensor("x", [TOK, D], dt.float32, kind="ExternalInput")
    aug_t = [
        nc.dram_tensor(f"aug{i}", [D, D], dt.bfloat16, kind="ExternalInput")
        for i in range(4)
    ]
    ex_t = nc.dram_tensor("ex", [D, W_EX], dt.bfloat16, kind="ExternalInput")
    cvec_t = nc.dram_tensor("cvec", [128, 16], dt.float32, kind="ExternalInput")
    lng_t = nc.dram_tensor("lng", [128, D], dt.float32, kind="ExternalInput")
    lnb_t = nc.dram_tensor("lnb", [128, D], dt.float32, kind="ExternalInput")
    y_t = nc.dram_tensor("y", [TOK, D], dt.float32, kind="ExternalOutput")

    RG = [[0, 1, 2, 3], [4, 5, 6, 7]]

    with tile.TileContext(nc) as tc, ExitStack() as stack:
        const = stack.enter_context(tc.tile_pool(name="const", bufs=1))
        dram = stack.enter_context(tc.tile_pool(name="dram", bufs=1, space="DRAM"))
        big = stack.enter_context(tc.tile_pool(name="big", bufs=1))

        agk_in = dram.tile([D, TOK], SC_DT, tag="agk_in")
        agk_pc = [
            dram.tile([4 * 256, TOK], SC_DT, tag=f"agk_pc{j}", name=f"agk_pc{j}")
            for j in range(4)
        ]
        agu_in = dram.tile([TOK, H], dt.bfloat16, tag="agu_in")
        agu_out = dram.tile([4 * TOK, H], dt.bfloat16, tag="agu_out")

        ident_bf = const.tile([128, 128], dt.bfloat16, tag="ident_bf")
        make_identity(nc, ident_bf[:])
        ident_f32 = const.tile([128, 128], dt.float32, tag="ident_f32")
        make_identity(nc, ident_f32[:])
        cvec = const.tile([128, 16], dt.float32, tag="cvec")
        nc.sync.dma_start(cvec[:], cvec_t[:])
        lng = const.tile([128, D], dt.float32, tag="lng")
        lnb = const.tile([128, D], dt.float32, tag="lnb")
        nc.sync.dma_start(lng[:], lng_t[:])
        nc.sync.dma_start(lnb[:], lnb_t[:])

        # persistent data tiles
        x32 = [big.tile([128, D], dt.float32, tag=f"x32_{m}", name=f"x32_{m}") for m in range(M4)]
        xbf = [big.tile([128, D], dt.bfloat16, tag=f"xbf_{m}", name=f"xbf_{m}") for m in range(M4)]
        xT = [big.tile([128, TOK], dt.bfloat16, tag=f"xT_{k}", name=f"xT_{k}") for k in range(K8)]
        qT = [big.tile([128, TOK], SC_DT, tag=f"qT_{k}", name=f"qT_{k}") for k in range(K8)]
        k3raw = [big.tile([128, D], dt.bfloat16, tag=f"k3_{m}", name=f"k3_{m}") for m in range(M4)]
        a3s = [big.tile([128, 1], dt.float32, tag=f"a3_{m}", name=f"a3_{m}") for m in range(M4)]
        b3s = [big.tile([128, 1], dt.float32, tag=f"b3_{m}", name=f"b3_{m}") for m in range(M4)]
        u_bf = [big.tile([128, H], dt.bfloat16, tag=f"u_{m}", name=f"u_{m}") for m in range(M4)]
        exsb = [big.tile([128, W_EX], dt.float32, tag=f"ex_{m}", name=f"ex_{m}") for m in range(M4)]
        v3acc = [big.tile([128, 1], dt.float32, tag=f"v3a_{m}", name=f"v3a_{m}") for m in range(M4)]
        mxs = [big.tile([128, 1], dt.float32, tag=f"mx_{m}", name=f"mx_{m}") for m in range(M4)]
        xxs = [big.tile([128, 1], dt.float32, tag=f"xx_{m}", name=f"xx_{m}") for m in range(M4)]
        mks3 = [big.tile([128, 1], dt.float32, tag=f"mk3_{m}", name=f"mk3_{m}") for m in range(M4)]
        kks3 = [big.tile([128, 1], dt.float32, tag=f"kk3_{m}", name=f"kk3_{m}") for m in range(M4)]
        xks3 = [big.tile([128, 1], dt.float32, tag=f"xk3_{m}", name=f"xk3_{m}") for m in range(M4)]
        nrw = big.tile([2, 2 * TOK], dt.float32, tag="nrw")
        aug3t = [
            big.tile([128, D], dt.bfloat16, tag=f"aug3_{k}", name=f"aug3w_{k}")
            for k in range(K8)
        ]

        for m in range(M4):
            nc.sync.dma_start(x32[m][:], x_t[ts(m, 128), :])
            nc.scalar.copy(xbf[m][:], x32[m][:])
            nc.vector.memset(v3acc[m][:], 0.0)
            nc.vector.tensor_reduce(mxs[m][:], x32[m][:], axis=mybir.AxisListType.X, op=ALU.add)
        for k in range(K8):
            nc.sync.dma_start(aug3t[k][:], aug_t[3][ts(k, 128), :])

        with (
            tc.tile_pool(name="wpool", bufs=16) as wpool,
            tc.tile_pool(name="qkpool", bufs=4) as qkpool,
            tc.tile_pool(name="scpool", bufs=24) as scpool,
            tc.tile_pool(name="scr", bufs=2) as scrpool,
            tc.tile_pool(name="ktloc", bufs=8) as ktlpool,
            tc.tile_pool(name="pp_proj", bufs=2, space="PSUM") as pp_proj,
            tc.tile_pool(name="pp_ex", bufs=2, space="PSUM") as pp_ex,
            tc.tile_pool(name="pp_t", bufs=2, space="PSUM") as pp_t,
        ):
            # x^T via PE transpose (bf16)
            for k in range(K8):
                pst = pp_t.tile([128, TOK], dt.bfloat16, tag="pst")
                for m in range(M4):
                    nc.tensor.transpose(
                        pst[:, ts(m, 128)], xbf[m][:, ts(k, 128)], ident_bf[:]
                    )
                nc.vector.tensor_copy(xT[k][:], pst[:])

            # extras matmul: all betas / v-scalars / u components at once
            ext = [wpool.tile([128, W_EX], dt.bfloat16, tag="ext", name=f"ext_{k}") for k in range(K8)]
            for k in range(K8):
                nc.sync.dma_start(ext[k][:], ex_t[ts(k, 128), :])
            for m in range(M4):
                pse = pp_ex.tile([128, W_EX], dt.float32, tag="pse")
                for k in range(K8):
                    nc.tensor.matmul(
                        pse[:], xT[k][:, ts(m, 128)], ext[k][:],
                        start=(k == 0), stop=(k == K8 - 1),
                    )
                nc.vector.tensor_copy(exsb[m][:], pse[:])

            qk_out = {}

            def scalar_chain(i, m, ps_beta_src, kx, rnorm):
                """beta, rk, rr from per-chunk scalars. Returns (rk, rr)."""
                ez = scpool.tile([128, 1], dt.float32, tag="sc", name=f"ez_{i}_{m}")
                nc.scalar.activation(
                    ez[:], ps_beta_src, AF.Exp, scale=-1.0, bias=cvec[:, i:i + 1]
                )
                ez1 = scpool.tile([128, 1], dt.float32, tag="sc", name=f"ez1_{i}_{m}")
                nc.vector.tensor_scalar_add(ez1[:], ez[:], 1.0)
                rsig = scpool.tile([128, 1], dt.float32, tag="sc", name=f"rs_{i}_{m}")
                nc.vector.reciprocal(rsig[:], ez1[:])
                rk = scpool.tile([128, 1], dt.float32, tag="sc", name=f"rk_{i}_{m}")
                nc.vector.tensor_scalar_mul(rk[:], kx[:], rnorm[:])
                rr = scpool.tile([128, 1], dt.float32, tag="sc", name=f"rr_{i}_{m}")
                nc.vector.tensor_scalar(rr[:], rsig[:], rnorm[:], 2.0, ALU.mult, ALU.mult)
                return rk, rr

            def rnorm_chain(i, m, ss):
                lnv = scpool.tile([128, 1], dt.float32, tag="sc", name=f"lnv_{i}_{m}")
                nc.scalar.activation(lnv[:], ss[:], AF.Ln)
                nrm = scpool.tile([128, 1], dt.float32, tag="sc", name=f"nrm_{i}_{m}")
                nc.scalar.activation(nrm[:], lnv[:], AF.Exp, scale=0.5)
                nrme = scpool.tile([128, 1], dt.float32, tag="sc", name=f"nrme_{i}_{m}")
                nc.vector.tensor_scalar_add(nrme[:], nrm[:], EPS)
                rnorm = scpool.tile([128, 1], dt.float32, tag="sc", name=f"rn_{i}_{m}")
                nc.vector.reciprocal(rnorm[:], nrme[:])
                return rnorm

            def delta_block(i):
                """dWk matmul + delta elementwise for aug i on all 4 chunks."""
                augt = [
                    wpool.tile([128, D], dt.bfloat16, tag="aug", name=f"aug_{i}_{k}")
                    for k in range(K8)
                ]
                for k in range(K8):
                    nc.sync.dma_start(augt[k][:], aug_t[i][ts(k, 128), :])
                outs = []
                for m in range(M4):
                    ps = pp_proj.tile([128, D], dt.float32, tag="ps_proj")
                    for k in range(K8):
                        for s0 in (0, 512):
                            nc.tensor.matmul(
                                ps[:, s0:s0 + 512], xT[k][:, ts(m, 128)],
                                augt[k][:, s0:s0 + 512],
                                start=(k == 0), stop=(k == K8 - 1),
                            )
                    ex = exsb[m]
                    scr = scrpool.tile([128, D], dt.bfloat16, tag="scr", name=f"scr_{i}_{m}")
                    ss = scpool.tile([128, 1], dt.float32, tag="sc", name=f"ss_{i}_{m}")
                    nc.scalar.activation(scr[:], ps[:], AF.Square, accum_out=ss[:])
                    kx = scpool.tile([128, 1], dt.float32, tag="sc", name=f"kx_{i}_{m}")
                    scr2 = scrpool.tile([128, D], dt.bfloat16, tag="scr", name=f"scr2_{i}_{m}")
                    nc.vector.scalar_tensor_tensor(
                        scr2[:], ps[:], 1.0, x32[m][:], ALU.mult, ALU.mult,
                        accum_out=kx[:],
                    )
                    rnorm = rnorm_chain(i, m, ss)
                    rk, rr = scalar_chain(i, m, ex[:, EX_DBW[i]:EX_DBW[i] + 1], kx, rnorm)
                    v = scpool.tile([128, 1], dt.float32, tag="sc", name=f"v_{i}_{m}")
                    nc.vector.tensor_scalar_add(
                        v[:], ex[:, EX_VW[i]:EX_VW[i] + 1], cvec[:, 4 + i:5 + i]
                    )
                    dv = scpool.tile([128, 1], dt.float32, tag="sc", name=f"dv_{i}_{m}")
                    nc.vector.tensor_tensor(dv[:], v[:], rk[:], ALU.subtract)
                    s = scpool.tile([128, 1], dt.float32, tag="sc", name=f"s_{i}_{m}")
                    nc.vector.tensor_tensor(s[:], dv[:], rr[:], ALU.mult)
                    if i in (0, 1):
                        o = qkpool.tile([128, D], dt.bfloat16, tag="qk", name=f"qk_{i}_{m}")
                        nc.vector.scalar_tensor_tensor(
                            o[:], ps[:], s[:], x32[m][:], ALU.mult, ALU.add
                        )
                        outs.append(o)
                    else:
                        # i == 2: u = A + s*B  (A/B live in the extras tile)
                        nc.vector.scalar_tensor_tensor(
                            u_bf[m][:], ex[:, EX_B:EX_B + H], s[:], ex[:, EX_A:EX_A + H],
                            ALU.mult, ALU.add,
                        )
                qk_out[i] = outs

            def delta3_chunk(m):
                """dWk3 matmul; elementwise on DVE from SBUF copy; LN moments."""
                psd = pp_proj.tile([128, D], dt.float32, tag="ps_proj")
                for k in range(K8):
                    for s0 in (0, 512):
                        nc.tensor.matmul(
                            psd[:, s0:s0 + 512], xT[k][:, ts(m, 128)],
                            aug3t[k][:, s0:s0 + 512],
                            start=(k == 0), stop=(k == K8 - 1),
                        )
                mka = scpool.tile([128, 1], dt.float32, tag="sc", name=f"mka_{m}")
                mkb = scpool.tile([128, 1], dt.float32, tag="sc", name=f"mkb_{m}")
                nc.vector.tensor_scalar(
                    k3raw[m][:, 0:512], psd[:, 0:512], 1.0, 0.0, ALU.mult,
                    ALU.add, accum_out=mka[:],
                )
                nc.vector.tensor_scalar(
                    k3raw[m][:, 512:1024], psd[:, 512:1024], 1.0, 0.0, ALU.mult,
                    ALU.add, accum_out=mkb[:],
                )
                nc.vector.tensor_tensor(mks3[m][:], mka[:], mkb[:], ALU.add)
                scr = scrpool.tile([128, D], dt.bfloat16, tag="scr", name=f"sc3r_{m}")
                nc.vector.scalar_tensor_tensor(
                    scr[:], k3raw[m][:], 1.0, k3raw[m][:], ALU.mult, ALU.mult,
                    accum_out=kks3[m][:],
                )
                scr2 = scrpool.tile([128, D], dt.bfloat16, tag="scr", name=f"sc3r2_{m}")
                nc.vector.scalar_tensor_tensor(
                    scr2[:], k3raw[m][:], 1.0, xbf[m][:], ALU.mult, ALU.mult,
                    accum_out=xks3[m][:],
                )
                rnorm = rnorm_chain(3, m, kks3[m])
                rk, rr = scalar_chain(3, m, exsb[m][:, EX_DBW[3]:EX_DBW[3] + 1], xks3[m], rnorm)
                nc.vector.tensor_copy(a3s[m][:], rr[:])
                nc.vector.tensor_tensor(b3s[m][:], rr[:], rk[:], ALU.mult)

            def transpose_to(src_tiles, dst_tiles):
                for k in range(K8):
                    pst = pp_t.tile([128, TOK], dt.bfloat16, tag="pst")
                    for m in range(M4):
                        nc.tensor.transpose(
                            pst[:, ts(m, 128)], src_tiles[m][:, ts(k, 128)], ident_bf[:]
                        )
                    nc.vector.tensor_copy(dst_tiles[k][:], pst[:])

            # ---- k path first so the AllGather starts early
            delta_block(1)
            ktloc = [ktlpool.tile([128, TOK], SC_DT, tag="ktloc", name=f"ktloc_{k}") for k in range(K8)]
            transpose_to(qk_out[1], ktloc)
            for k in range(K8):
                nc.sync.dma_start(agk_in[ts(k, 128), :], ktloc[k][:])
            nc.gpsimd.collective_compute(
                "AllGather", ALU.bypass, ins=[agk_in[0:256, :]], outs=[agk_pc[0][:]],
                replica_groups=RG,
            )
            delta_block(2)
            for m in range(M4):
                nc.sync.dma_start(agu_in[ts(m, 128), :], u_bf[m][:])
            nc.gpsimd.collective_compute(
                "AllGather", ALU.bypass, ins=[agu_in[:]], outs=[agu_out[:]],
                replica_groups=RG,
            )
            for j in range(1, 4):
                nc.gpsimd.collective_compute(
                    "AllGather", ALU.bypass,
                    ins=[agk_in[256 * j:256 * (j + 1), :]], outs=[agk_pc[j][:]],
                    replica_groups=RG,
                )
            delta_block(0)
            transpose_to(qk_out[0], qT)
            for m in range(M4):
                delta3_chunk(m)
                xsq = scrpool.tile([128, D], dt.bfloat16, tag="scr", name=f"xsq_{m}")
                nc.scalar.activation(xsq[:], x32[m][:], AF.Square, accum_out=xxs[m][:])

        # ---------------- attention ----------------
        with (
            tc.tile_pool(name="attn_sb", bufs=1) as attn_sb,
            tc.tile_pool(name="epool", bufs=4) as epool,
            tc.tile_pool(name="fin", bufs=2) as fin,
            tc.tile_pool(name="pp_sc", bufs=3, space="PSUM") as pp_sc,
            tc.tile_pool(name="pp_nr", bufs=2, space="PSUM") as pp_nr,
        ):
            kT = [attn_sb.tile([128, S], SC_DT, tag=f"kT_{k}", name=f"kTsb_{k}") for k in range(K8)]
            for k in range(K8):
                src = agk_pc[k // 2][:].rearrange("(c d) t -> d c t", c=4)[ts(k % 2, 128), :, :]
                dst = kT[k][:].rearrange("p (c t) -> p c t", c=4)
                nc.gpsimd.dma_start(dst, src)
            uext = attn_sb.tile([128, NKC, H, 2], dt.bfloat16, tag="uext")
            nc.vector.memset(uext[:], 1.0)
            u_all = attn_sb.tile([128, NKC, H], dt.bfloat16, tag="u_all")
            nc.gpsimd.dma_start(
                u_all[:], agu_out[:].rearrange("(kc p) h -> p kc h", p=128)
            )
            nc.vector.tensor_copy(uext[:, :, :, 0], u_all[:])

            SCALE = float(HD) ** -0.5

            for hp in range(K8):         # 8 head pairs; pair hp = heads 2hp, 2hp+1
                nr_ps = pp_nr.tile([128, TOK], dt.float32, tag="nr")
                hA, hB = 2 * hp, 2 * hp + 1
                for kc in range(NKC):
                    ps2 = pp_sc.tile([128, 2, TOK], dt.float32, tag="sc2")
                    nc.tensor.matmul(
                        ps2[:, 0, :], kT[hp][0:64, ts(kc, 128)], qT[hp][0:64, :],
                        start=True, stop=True, tile_position=(0, 0),
                    )
                    nc.tensor.matmul(
                        ps2[:, 1, :], kT[hp][64:128, ts(kc, 128)], qT[hp][64:128, :],
                        start=True, stop=True, tile_position=(64, 0),
                    )
                    E = epool.tile([128, 2, TOK], dt.bfloat16, tag="E")
                    nc.scalar.activation(E[:], ps2[:], AF.Exp, scale=SCALE)
                    nc.tensor.matmul(
                        nr_ps[0:2, :], uext[:, kc, hA, :], E[:, 0, :],
                        start=(kc == 0), stop=(kc == NKC - 1),
                        tile_position=(0, 0),
                    )
                    nc.tensor.matmul(
                        nr_ps[32:34, :], uext[:, kc, hB, :], E[:, 1, :],
                        start=(kc == 0), stop=(kc == NKC - 1),
                        tile_position=(0, 32),
                    )
                # stage the pair's n/r rows and fold into v3acc (overlaps attention)
                for j in range(2):
                    nc.vector.tensor_copy(
                        nrw[0:2, j * TOK:(j + 1) * TOK], nr_ps[32 * j:32 * j + 2, :]
                    )
                for m in range(M4):
                    psT = pp_sc.tile([128, 2, TOK], dt.float32, tag="sc2")
                    for j in range(2):
                        nc.tensor.transpose(
                            psT[:, 0, 2 * j:2 * j + 2],
                            nrw[0:2, j * TOK + 128 * m: j * TOK + 128 * (m + 1)],
                            ident_f32[0:2, 0:2],
                        )
                    nrT = fin.tile([128, 4], dt.float32, tag="nrT", name=f"nrT_{hp}_{m}")
                    nc.vector.tensor_copy(nrT[:], psT[:, 0, 0:4])
                    rec = fin.tile([128, 2], dt.float32, tag="rec", name=f"rec_{hp}_{m}")
                    nc.vector.reciprocal(rec[:], nrT[:, 1:4:2])
                    prod = fin.tile([128, 2], dt.float32, tag="prod", name=f"pr_{hp}_{m}")
                    nc.vector.tensor_tensor(prod[:], nrT[:, 0:4:2], rec[:], ALU.mult)
                    pv = fin.tile([128, 1], dt.float32, tag="pv", name=f"pv_{hp}_{m}")
                    nc.vector.tensor_reduce(pv[:], prod[:], axis=mybir.AxisListType.X, op=ALU.add)
                    nc.vector.tensor_tensor(v3acc[m][:], v3acc[m][:], pv[:], ALU.add)

            # ---- final delta + layernorm (stats from precomputed moments)
            for m in range(M4):
                v3 = fin.tile([128, 1], dt.float32, tag="v3", name=f"v3_{m}")
                nc.vector.tensor_scalar_add(v3[:], v3acc[m][:], cvec[:, 7:8])
                s3 = fin.tile([128, 1], dt.float32, tag="s3", name=f"s3_{m}")
                nc.vector.tensor_scalar_mul(s3[:], v3[:], a3s[m][:])
                nc.vector.tensor_tensor(s3[:], s3[:], b3s[m][:], ALU.subtract)
                # mu = (sum_x + s3*sum_k)/D
                mu = fin.tile([128, 1], dt.float32, tag="mu", name=f"mu_{m}")
                nc.vector.tensor_scalar_mul(mu[:], s3[:], mks3[m][:])
                nc.vector.tensor_tensor(mu[:], mu[:], mxs[m][:], ALU.add)
                nc.vector.tensor_scalar_mul(mu[:], mu[:], 1.0 / D)
                # E[y^2] = (xx + 2 s3 xk + s3^2 kk)/D ; var = E[y^2] - mu^2
                t1 = fin.tile([128, 1], dt.float32, tag="t1", name=f"t1_{m}")
                nc.vector.tensor_scalar_mul(t1[:], s3[:], kks3[m][:])
                t2 = fin.tile([128, 1], dt.float32, tag="t2", name=f"t2_{m}")
                nc.vector.tensor_scalar(t2[:], xks3[m][:], 2.0, None, ALU.mult)
                nc.vector.tensor_tensor(t2[:], t2[:], t1[:], ALU.add)
                nc.vector.tensor_scalar_mul(t2[:], t2[:], s3[:])
                nc.vector.tensor_tensor(t2[:], t2[:], xxs[m][:], ALU.add)
                var = fin.tile([128, 1], dt.float32, tag="var", name=f"var_{m}")
                nc.vector.tensor_scalar_mul(var[:], t2[:], 1.0 / D)
                mu2 = fin.tile([128, 1], dt.float32, tag="mu2", name=f"mu2_{m}")
                nc.vector.tensor_tensor(mu2[:], mu[:], mu[:], ALU.mult)
                nc.vector.tensor_tensor(var[:], var[:], mu2[:], ALU.subtract)
                nc.vector.tensor_scalar_add(var[:], var[:], LN_EPS)
                lnv2 = fin.tile([128, 1], dt.float32, tag="lnv2", name=f"lnv2_{m}")
                nc.scalar.activation(lnv2[:], var[:], AF.Ln)
                rstd = fin.tile([128, 1], dt.float32, tag="rstd", name=f"rstd_{m}")
                nc.scalar.activation(rstd[:], lnv2[:], AF.Exp, scale=-0.5)
                s3r = fin.tile([128, 1], dt.float32, tag="s3r", name=f"s3r_{m}")
                nc.vector.tensor_scalar_mul(s3r[:], s3[:], rstd[:])
                # yn = (x - mu)*rstd + k3raw*(s3*rstd) ; out = yn*g + b
                yn = fin.tile([128, D], dt.float32, tag="yn", name=f"yn_{m}")
                nc.vector.tensor_scalar(yn[:], x32[m][:], mu[:], rstd[:], ALU.subtract, ALU.mult)
                nc.vector.scalar_tensor_tensor(
                    yn[:], k3raw[m][:], s3r[:], yn[:], ALU.mult, ALU.add
                )
                yg = fin.tile([128, D], dt.float32, tag="yg", name=f"yg_{m}")
                nc.vector.tensor_tensor(yg[:], yn[:], lng[:], ALU.mult)
                nc.vector.tensor_tensor(yg[:], yg[:], lnb[:], ALU.add)
                nc.sync.dma_start(y_t[ts(m, 128), :], yg[:])

    _split_multi_waits(nc)
    nc.finalize()
    return nc


def _host_prep(inputs):
    """Precompute augmented weights and constants; returns per-core in_maps."""
    f32 = np.float32
    x = np.asarray(inputs["x"], f32)
    Wq, bq = np.asarray(inputs["Wq"], f32), np.asarray(inputs["bq"], f32)
    Wk, bk = np.asarray(inputs["Wk"], f32), np.asarray(inputs["bk"], f32)
    Wv, bv = np.asarray(inputs["Wv"], f32), np.asarray(inputs["bv"], f32)
    Wo, bo = np.asarray(inputs["Wo"], f32), np.asarray(inputs["bo"], f32)
    dWk, dbw = np.asarray(inputs["dWk"], f32), np.asarray(inputs["dbw"], f32)
    dbb, dWv = np.asarray(inputs["dbb"], f32), np.asarray(inputs["dWv"], f32)
    dbv = np.asarray(inputs["dbv"], f32)
    ln_g, ln_b = np.asarray(inputs["ln_g"], f32), np.asarray(inputs["ln_b"], f32)

    w = Wo @ dWv[3]                                   # (D,)
    Wu = np.zeros((D, H), f32)
    for h in range(H):
        Wu[h * HD:(h + 1) * HD, h] = w[h * HD:(h + 1) * HD]
    Bu = dWk[2] @ Wu                                  # (D, H)

    vw = [Wq @ dWv[0], Wk @ dWv[1], Wv @ dWv[2]]
    vc = [float(bq @ dWv[0] + dbv[0]), float(bk @ dWv[1] + dbv[1]),
          float(bv @ dWv[2] + dbv[2])]
    c3 = float(bo @ dWv[3] + dbv[3])

    bf = ml_dtypes.bfloat16
    augs = [np.ascontiguousarray(dWk[i]).astype(bf) for i in range(4)]
    ex = np.zeros((D, W_EX), f32)
    for i in range(4):
        ex[:, EX_DBW[i]] = dbw[i]
    for i in range(3):
        ex[:, EX_VW[i]] = vw[i]
    ex[:, EX_A:EX_A + H] = Wu
    ex[:, EX_B:EX_B + H] = Bu
    ex = ex.astype(bf)

    cvec = np.zeros((128, 16), f32)
    for i in range(4):
        cvec[:, i] = -dbb[i]
    for i in range(3):
        cvec[:, 4 + i] = vc[i]
    cvec[:, 7] = c3

    lng = np.broadcast_to(ln_g[None, :], (128, D)).copy()
    lnb = np.broadcast_to(ln_b[None, :], (128, D)).copy()

    xf = x.reshape(B * S, D)
    in_maps = []
    for c in range(N_CORES):
        m = {
            "x": np.ascontiguousarray(xf[c * TOK:(c + 1) * TOK]),
            "ex": ex, "cvec": cvec, "lng": lng, "lnb": lnb,
        }
        for i in range(4):
            m[f"aug{i}"] = augs[i]
        in_maps.append(m)
    return in_maps


def kernel(**inputs):
    global LAST_RESULTS
    if "nc" not in _CACHE:
        _CACHE["nc"] = _build_program()
    nc = _CACHE["nc"]
    in_maps = _host_prep(inputs)
    res = run_bass_kernel_spmd(nc, in_maps, core_ids=list(range(N_CORES)))
    LAST_RESULTS = res
    out = np.concatenate(
        [res.results[c]["y"] for c in range(N_CORES)], axis=0
    ).reshape(B, S, D)
    return out.astype(np.float32)


# revision 24
# speedup vs baseline: 1.0744x; 1.0348x over previous
"""DeltaAttention Trainium2 kernel — 8-core SPMD via bass/Tile.

Math (per reference): 4 DeltaResidualBlocks (d_v=1) wrapped around MHA.
Because each delta block consumes its v_in only through the scalar
projection v_in @ dWv[i], the Wq/Wk/Wv/Wo matmuls collapse into single
extra columns of the dWk matmuls (precomputed on host), and attn@v
collapses to 2 output columns per head:
    n_h[q] = E_h[q,:] @ u_h,  r_h[q] = E_h[q,:] @ 1,  u_h = v_h @ w_h
    v3[q]  = sum_h n_h/r_h + const,   w = Wo @ dWv[3]
Sharding: 512 query tokens per core; k^T and u are AllGathered within
each 4-core batch group.  LayerNorm statistics are computed from
precomputed moments of x and k3_raw (y = x + s3*k3_raw).
"""

import os
from contextlib import ExitStack

import numpy as np
import ml_dtypes

import concourse.bass as bass
import concourse.mybir as mybir
import concourse.tile as tile
from concourse.bass_utils import run_bass_kernel_spmd
from concourse.masks import make_identity

dt = mybir.dt
AF = mybir.ActivationFunctionType
ALU = mybir.AluOpType
ts = bass.ts

N_CORES = 8
B, S, D, H = 2, 2048, 1024, 16
HD = D // H
TOK = (B * S) // N_CORES          # 512 query tokens per core
M4 = TOK // 128                   # 4 token chunks
K8 = D // 128                     # 8 feature chunks
NKC = S // 128                    # 16 key chunks per batch
EPS = 1e-8
LN_EPS = 1e-5

# extras matmul columns: [dbw0,vw0, dbw1,vw1, dbw2,vw2, Wu(16), Bu(16), dbw3]
W_EX = 39
EX_DBW = [0, 2, 4, 38]
EX_VW = [1, 3, 5]
EX_A = 6      # 6..22  = x @ Wu
EX_B = 22     # 22..38 = x @ dWk2 @ Wu

# dtype of q^T / k^T used by the scores matmul (fp8 halves the AllGather)
SC_DT = dt.float8e4 if os.environ.get("DELTA_SCORES_FP8", "1") == "1" else dt.bfloat16

LAST_RESULTS = None
_CACHE = {}


def _split_multi_waits(nc, max_waits=1):
    """walrus (CoreV3) only encodes one sync wait per instruction; Tile's
    final drain can carry several. Hoist extras onto preceding NoOps."""
    n_fixed = 0
    for f in nc.m.functions:
        for blk in f.blocks:
            new_insts = []
            for inst in blk.instructions:
                si = inst.sync_info
                waits = list(si.on_wait) if (si and si.on_wait) else []
                if len(waits) > max_waits:
                    head, tail = waits[:-max_waits], waits[-max_waits:]
                    for j, w in enumerate(head):
                        nop = mybir.InstNoOp(
                            name=f"{inst.name}_waitsplit_{j}",
                            engine=inst.engine,
                            ins=[],
                            outs=[],
                            sync_info=mybir.SyncInfo(on_wait=[w], on_update=[]),
                        )
                        nc.register_instruction(nop)
                        new_insts.append(nop)
                        n_fixed += 1
                    si.on_wait = tail
                new_insts.append(inst)
            blk.instructions[:] = new_insts
    return n_fixed


def _build_program():
    nc = bass.Bass(num_devices=N_CORES)

    x_t = nc.dram_tensor("x", [TOK, D], dt.float32, kind="ExternalInput")
    aug_t = [
        nc.dram_tensor(f"aug{i}", [D, D], dt.bfloat16, kind="ExternalInput")
        for i in range(4)
    ]
    ex_t = nc.dram_tensor("ex", [D, W_EX], dt.bfloat16, kind="ExternalInput")
    cvec_t = nc.dram_tensor("cvec", [128, 16], dt.float32, kind="ExternalInput")
    lng_t = nc.dram_tensor("lng", [128, D], dt.float32, kind="ExternalInput")
    lnb_t = nc.dram_tensor("lnb", [128, D], dt.float32, kind="ExternalInput")
    y_t = nc.dram_tensor("y", [TOK, D], dt.float32, kind="ExternalOutput")

    RG = [[0, 1, 2, 3], [4, 5, 6, 7]]

    with tile.TileContext(nc) as tc, ExitStack() as stack:
        const = stack.enter_context(tc.tile_pool(name="const", bufs=1))
        dram = stack.enter_context(tc.tile_pool(name="dram", bufs=1, space="DRAM"))
        big = stack.enter_context(tc.tile_pool(name="big", bufs=1))

        agk_in = dram.tile([D, TOK], SC_DT, tag="agk_in")
        agk_pc = [
            dram.tile([4 * 256, TOK], SC_DT, tag=f"agk_pc{j}", name=f"agk_pc{j}")
            for j in range(4)
        ]
        agu_in = dram.tile([TOK, H], dt.bfloat16, tag="agu_in")
        agu_out = dram.tile([4 * TOK, H], dt.bfloat16, tag="agu_out")

        ident_bf = const.tile([128, 128], dt.bfloat16, tag="ident_bf")
        make_identity(nc, ident_bf[:])
        ident_f32 = const.tile([128, 128], dt.float32, tag="ident_f32")
        make_identity(nc, ident_f32[:])
        cvec = const.tile([128, 16], dt.float32, tag="cvec")
        nc.sync.dma_start(cvec[:], cvec_t[:])
        lng = const.tile([128, D], dt.float32, tag="lng")
        lnb = const.tile([128, D], dt.float32, tag="lnb")
        nc.sync.dma_start(lng[:], lng_t[:])
        nc.sync.dma_start(lnb[:], lnb_t[:])

        # persistent data tiles
        x32 = [big.tile([128, D], dt.float32, tag=f"x32_{m}", name=f"x32_{m}") for m in range(M4)]
        xbf = [big.tile([128, D], dt.bfloat16, tag=f"xbf_{m}", name=f"xbf_{m}") for m in range(M4)]
        xT = [big.tile([128, TOK], dt.bfloat16, tag=f"xT_{k}", name=f"xT_{k}") for k in range(K8)]
        qT = [big.tile([128, TOK], SC_DT, tag=f"qT_{k}", name=f"qT_{k}") for k in range(K8)]
        k3raw = [big.tile([128, D], dt.bfloat16, tag=f"k3_{m}", name=f"k3_{m}") for m in range(M4)]
        a3s = [big.tile([128, 1], dt.float32, tag=f"a3_{m}", name=f"a3_{m}") for m in range(M4)]
        b3s = [big.tile([128, 1], dt.float32, tag=f"b3_{m}", name=f"b3_{m}") for m in range(M4)]
        u_bf = [big.tile([128, H], dt.bfloat16, tag=f"u_{m}", name=f"u_{m}") for m in range(M4)]
        exsb = [big.tile([128, W_EX], dt.float32, tag=f"ex_{m}", name=f"ex_{m}") for m in range(M4)]
        v3acc = [big.tile([128, 1], dt.float32, tag=f"v3a_{m}", name=f"v3a_{m}") for m in range(M4)]
        mxs = [big.tile([128, 1], dt.float32, tag=f"mx_{m}", name=f"mx_{m}") for m in range(M4)]
        xxs = [big.tile([128, 1], dt.float32, tag=f"xx_{m}", name=f"xx_{m}") for m in range(M4)]
        mks3 = [big.tile([128, 1], dt.float32, tag=f"mk3_{m}", name=f"mk3_{m}") for m in range(M4)]
        kks3 = [big.tile([128, 1], dt.float32, tag=f"kk3_{m}", name=f"kk3_{m}") for m in range(M4)]
        xks3 = [big.tile([128, 1], dt.float32, tag=f"xk3_{m}", name=f"xk3_{m}") for m in range(M4)]
        nrw = big.tile([2, 2 * TOK], dt.float32, tag="nrw")
        aug3t = [
            big.tile([128, D], dt.bfloat16, tag=f"aug3_{k}", name=f"aug3w_{k}")
            for k in range(K8)
        ]

        for m in range(M4):
            nc.sync.dma_start(x32[m][:], x_t[ts(m, 128), :])
            nc.scalar.copy(xbf[m][:], x32[m][:])
            nc.vector.memset(v3acc[m][:], 0.0)
            nc.vector.tensor_reduce(mxs[m][:], x32[m][:], axis=mybir.AxisListType.X, op=ALU.add)
        for k in range(K8):
            nc.sync.dma_start(aug3t[k][:], aug_t[3][ts(k, 128), :])

        with (
            tc.tile_pool(name="wpool", bufs=16) as wpool,
            tc.tile_pool(name="qkpool", bufs=4) as qkpool,
            tc.tile_pool(name="scpool", bufs=24) as scpool,
            tc.tile_pool(name="scr", bufs=2) as scrpool,
            tc.tile_pool(name="ktloc", bufs=8) as ktlpool,
            tc.tile_pool(name="pp_proj", bufs=2, space="PSUM") as pp_proj,
            tc.tile_pool(name="pp_ex", bufs=2, space="PSUM") as pp_ex,
            tc.tile_pool(name="pp_t", bufs=2, space="PSUM") as pp_t,
        ):
            # x^T via PE transpose (bf16)
            for k in range(K8):
                pst = pp_t.tile([128, TOK], dt.bfloat16, tag="pst")
                for m in range(M4):
                    nc.tensor.transpose(
                        pst[:, ts(m, 128)], xbf[m][:, ts(k, 128)], ident_bf[:]
                    )
                nc.vector.tensor_copy(xT[k][:], pst[:])

            # extras matmul: all betas / v-scalars / u components at once
            ext = [wpool.tile([128, W_EX], dt.bfloat16, tag="ext", name=f"ext_{k}") for k in range(K8)]
            for k in range(K8):
                nc.sync.dma_start(ext[k][:], ex_t[ts(k, 128), :])
            for m in range(M4):
                pse = pp_ex.tile([128, W_EX], dt.float32, tag="pse")
                for k in range(K8):
                    nc.tensor.matmul(
                        pse[:], xT[k][:, ts(m, 128)], ext[k][:],
                        start=(k == 0), stop=(k == K8 - 1),
                    )
                nc.vector.tensor_copy(exsb[m][:], pse[:])

            qk_out = {}

            def scalar_chain(i, m, ps_beta_src, kx, rnorm):
                """beta, rk, rr from per-chunk scalars. Returns (rk, rr)."""
                ez = scpool.tile([128, 1], dt.float32, tag="sc", name=f"ez_{i}_{m}")
                nc.scalar.activation(
                    ez[:], ps_beta_src, AF.Exp, scale=-1.0, bias=cvec[:, i:i + 1]
                )
                ez1 = scpool.tile([128, 1], dt.float32, tag="sc", name=f"ez1_{i}_{m}")
                nc.vector.tensor_scalar_add(ez1[:], ez[:], 1.0)
                rsig = scpool.tile([128, 1], dt.float32, tag="sc", name=f"rs_{i}_{m}")
                nc.vector.reciprocal(rsig[:], ez1[:])
                rk = scpool.tile([128, 1], dt.float32, tag="sc", name=f"rk_{i}_{m}")
                nc.vector.tensor_scalar_mul(rk[:], kx[:], rnorm[:])
                rr = scpool.tile([128, 1], dt.float32, tag="sc", name=f"rr_{i}_{m}")
                nc.vector.tensor_scalar(rr[:], rsig[:], rnorm[:], 2.0, ALU.mult, ALU.mult)
                return rk, rr

            def rnorm_chain(i, m, ss):
                lnv = scpool.tile([128, 1], dt.float32, tag="sc", name=f"lnv_{i}_{m}")
                nc.scalar.activation(lnv[:], ss[:], AF.Ln)
                nrm = scpool.tile([128, 1], dt.float32, tag="sc", name=f"nrm_{i}_{m}")
                nc.scalar.activation(nrm[:], lnv[:], AF.Exp, scale=0.5)
                nrme = scpool.tile([128, 1], dt.float32, tag="sc", name=f"nrme_{i}_{m}")
                nc.vector.tensor_scalar_add(nrme[:], nrm[:], EPS)
                rnorm = scpool.tile([128, 1], dt.float32, tag="sc", name=f"rn_{i}_{m}")
                nc.vector.reciprocal(rnorm[:], nrme[:])
                return rnorm

            def delta_block(i):
                """dWk matmul + delta elementwise for aug i on all 4 chunks."""
                augt = [
                    wpool.tile([128, D], dt.bfloat16, tag="aug", name=f"aug_{i}_{k}")
                    for k in range(K8)
                ]
                for k in range(K8):
                    nc.sync.dma_start(augt[k][:], aug_t[i][ts(k, 128), :])
                outs = []
                for m in range(M4):
                    ps = pp_proj.tile([128, D], dt.float32, tag="ps_proj")
                    for k in range(K8):
                        for s0 in (0, 512):
                            nc.tensor.matmul(
                                ps[:, s0:s0 + 512], xT[k][:, ts(m, 128)],
                                augt[k][:, s0:s0 + 512],
                                start=(k == 0), stop=(k == K8 - 1),
                            )
                    ex = exsb[m]
                    scr = scrpool.tile([128, D], dt.bfloat16, tag="scr", name=f"scr_{i}_{m}")
                    ss = scpool.tile([128, 1], dt.float32, tag="sc", name=f"ss_{i}_{m}")
                    nc.scalar.activation(scr[:], ps[:], AF.Square, accum_out=ss[:])
                    kx = scpool.tile([128, 1], dt.float32, tag="sc", name=f"kx_{i}_{m}")
                    scr2 = scrpool.tile([128, D], dt.bfloat16, tag="scr", name=f"scr2_{i}_{m}")
                    nc.vector.scalar_tensor_tensor(
                        scr2[:], ps[:], 1.0, x32[m][:], ALU.mult, ALU.mult,
                        accum_out=kx[:],
                    )
                    rnorm = rnorm_chain(i, m, ss)
                    rk, rr = scalar_chain(i, m, ex[:, EX_DBW[i]:EX_DBW[i] + 1], kx, rnorm)
                    v = scpool.tile([128, 1], dt.float32, tag="sc", name=f"v_{i}_{m}")
                    nc.vector.tensor_scalar_add(
                        v[:], ex[:, EX_VW[i]:EX_VW[i] + 1], cvec[:, 4 + i:5 + i]
                    )
                    dv = scpool.tile([128, 1], dt.float32, tag="sc", name=f"dv_{i}_{m}")
                    nc.vector.tensor_tensor(dv[:], v[:], rk[:], ALU.subtract)
                    s = scpool.tile([128, 1], dt.float32, tag="sc", name=f"s_{i}_{m}")
                    nc.vector.tensor_tensor(s[:], dv[:], rr[:], ALU.mult)
                    if i in (0, 1):
                        o = qkpool.tile([128, D], dt.bfloat16, tag="qk", name=f"qk_{i}_{m}")
                        nc.vector.scalar_tensor_tensor(
                            o[:], ps[:], s[:], x32[m][:], ALU.mult, ALU.add
                        )
                        outs.append(o)
                    else:
                        # i == 2: u = A + s*B  (A/B live in the extras tile)
                        nc.vector.scalar_tensor_tensor(
                            u_bf[m][:], ex[:, EX_B:EX_B + H], s[:], ex[:, EX_A:EX_A + H],
                            ALU.mult, ALU.add,
                        )
                qk_out[i] = outs

            def delta3_chunk(m):
                """dWk3 matmul; elementwise on DVE from SBUF copy; LN moments."""
                psd = pp_proj.tile([128, D], dt.float32, tag="ps_proj")
                for k in range(K8):
                    for s0 in (0, 512):
                        nc.tensor.matmul(
                            psd[:, s0:s0 + 512], xT[k][:, ts(m, 128)],
                            aug3t[k][:, s0:s0 + 512],
                            start=(k == 0), stop=(k == K8 - 1),
                        )
                mka = scpool.tile([128, 1], dt.float32, tag="sc", name=f"mka_{m}")
                mkb = scpool.tile([128, 1], dt.float32, tag="sc", name=f"mkb_{m}")
                nc.vector.tensor_scalar(
                    k3raw[m][:, 0:512], psd[:, 0:512], 1.0, 0.0, ALU.mult,
                    ALU.add, accum_out=mka[:],
                )
                nc.vector.tensor_scalar(
                    k3raw[m][:, 512:1024], psd[:, 512:1024], 1.0, 0.0, ALU.mult,
                    ALU.add, accum_out=mkb[:],
                )
                nc.vector.tensor_tensor(mks3[m][:], mka[:], mkb[:], ALU.add)
                scr = scrpool.tile([128, D], dt.bfloat16, tag="scr", name=f"sc3r_{m}")
                nc.vector.scalar_tensor_tensor(
                    scr[:], k3raw[m][:], 1.0, k3raw[m][:], ALU.mult, ALU.mult,
                    accum_out=kks3[m][:],
                )
                scr2 = scrpool.tile([128, D], dt.bfloat16, tag="scr", name=f"sc3r2_{m}")
                nc.vector.scalar_tensor_tensor(
                    scr2[:], k3raw[m][:], 1.0, xbf[m][:], ALU.mult, ALU.mult,
                    accum_out=xks3[m][:],
                )
                rnorm = rnorm_chain(3, m, kks3[m])
                rk, rr = scalar_chain(3, m, exsb[m][:, EX_DBW[3]:EX_DBW[3] + 1], xks3[m], rnorm)
                nc.vector.tensor_copy(a3s[m][:], rr[:])
                nc.vector.tensor_tensor(b3s[m][:], rr[:], rk[:], ALU.mult)

            def transpose_to(src_tiles, dst_tiles):
                for k in range(K8):
                    pst = pp_t.tile([128, TOK], dt.bfloat16, tag="pst")
                    for m in range(M4):
                        nc.tensor.transpose(
                            pst[:, ts(m, 128)], src_tiles[m][:, ts(k, 128)], ident_bf[:]
                        )
                    nc.vector.tensor_copy(dst_tiles[k][:], pst[:])

            # ---- k path first so the AllGather starts early
            delta_block(1)
            ktloc = [ktlpool.tile([128, TOK], SC_DT, tag="ktloc", name=f"ktloc_{k}") for k in range(K8)]
            transpose_to(qk_out[1], ktloc)
            for k in range(K8):
                nc.sync.dma_start(agk_in[ts(k, 128), :], ktloc[k][:])
            nc.gpsimd.collective_compute(
                "AllGather", ALU.bypass, ins=[agk_in[0:256, :]], outs=[agk_pc[0][:]],
                replica_groups=RG,
            )
            delta_block(2)
            for m in range(M4):
                nc.sync.dma_start(agu_in[ts(m, 128), :], u_bf[m][:])
            nc.gpsimd.collective_compute(
                "AllGather", ALU.bypass, ins=[agu_in[:]], outs=[agu_out[:]],
                replica_groups=RG,
            )
            for j in range(1, 4):
                nc.gpsimd.collective_compute(
                    "AllGather", ALU.bypass,
                    ins=[agk_in[256 * j:256 * (j + 1), :]], outs=[agk_pc[j][:]],
                    replica_groups=RG,
                )
            delta_block(0)
            transpose_to(qk_out[0], qT)
            for m in range(M4):
                delta3_chunk(m)
                xsq = scrpool.tile([128, D], dt.bfloat16, tag="scr", name=f"xsq_{m}")
                nc.scalar.activation(xsq[:], x32[m][:], AF.Square, accum_out=xxs[m][:])

        # ---------------- attention ----------------
        with (
            tc.tile_pool(name="attn_sb", bufs=1) as attn_sb,
            tc.tile_pool(name="epool", bufs=4) as epool,
            tc.tile_pool(name="fin", bufs=2) as fin,
            tc.tile_pool(name="pp_sc", bufs=3, space="PSUM") as pp_sc,
            tc.tile_pool(name="pp_nr", bufs=2, space="PSUM") as pp_nr,
        ):
            kT = [attn_sb.tile([128, S], SC_DT, tag=f"kT_{k}", name=f"kTsb_{k}") for k in range(K8)]
            for k in range(K8):
                src = agk_pc[k // 2][:].rearrange("(c d) t -> d c t", c=4)[ts(k % 2, 128), :, :]
                dst = kT[k][:].rearrange("p (c t) -> p c t", c=4)
                nc.sync.dma_start(dst, src)
            uext = attn_sb.tile([128, NKC, H, 2], dt.bfloat16, tag="uext")
            nc.vector.memset(uext[:], 1.0)
            u_all = attn_sb.tile([128, NKC, H], dt.bfloat16, tag="u_all")
            nc.sync.dma_start(
                u_all[:], agu_out[:].rearrange("(kc p) h -> p kc h", p=128)
            )
            nc.vector.tensor_copy(uext[:, :, :, 0], u_all[:])

            SCALE = float(HD) ** -0.5

            for hp in range(K8):         # 8 head pairs; pair hp = heads 2hp, 2hp+1
                nr_ps = pp_nr.tile([128, TOK], dt.float32, tag="nr")
                hA, hB = 2 * hp, 2 * hp + 1
                for kc in range(NKC):
                    ps2 = pp_sc.tile([128, 2, TOK], dt.float32, tag="sc2")
                    nc.tensor.matmul(
                        ps2[:, 0, :], kT[hp][0:64, ts(kc, 128)], qT[hp][0:64, :],
                        start=True, stop=True, tile_position=(0, 0),
                    )
                    nc.tensor.matmul(
                        ps2[:, 1, :], kT[hp][64:128, ts(kc, 128)], qT[hp][64:128, :],
                        start=True, stop=True, tile_position=(64, 0),
                    )
                    E = epool.tile([128, 2, TOK], dt.bfloat16, tag="E")
                    nc.scalar.activation(E[:], ps2[:], AF.Exp, scale=SCALE)
                    nc.tensor.matmul(
                        nr_ps[0:2, :], uext[:, kc, hA, :], E[:, 0, :],
                        start=(kc == 0), stop=(kc == NKC - 1),
                        tile_position=(0, 0),
                    )
                    nc.tensor.matmul(
                        nr_ps[32:34, :], uext[:, kc, hB, :], E[:, 1, :],
                        start=(kc == 0), stop=(kc == NKC - 1),
                        tile_position=(0, 32),
                    )
                # stage the pair's n/r rows and fold into v3acc (overlaps attention)
                for j in range(2):
                    nc.vector.tensor_copy(
                        nrw[0:2, j * TOK:(j + 1) * TOK], nr_ps[32 * j:32 * j + 2, :]
                    )
                for m in range(M4):
                    psT = pp_sc.tile([128, 2, TOK], dt.float32, tag="sc2")
                    for j in range(2):
                        nc.tensor.transpose(
                            psT[:, 0, 2 * j:2 * j + 2],
                            nrw[0:2, j * TOK + 128 * m: j * TOK + 128 * (m + 1)],
                            ident_f32[0:2, 0:2],
                        )
                    nrT = fin.tile([128, 4], dt.float32, tag="nrT", name=f"nrT_{hp}_{m}")
                    nc.vector.tensor_copy(nrT[:], psT[:, 0, 0:4])
                    rec = fin.tile([128, 2], dt.float32, tag="rec", name=f"rec_{hp}_{m}")
                    nc.vector.reciprocal(rec[:], nrT[:, 1:4:2])
                    prod = fin.tile([128, 2], dt.float32, tag="prod", name=f"pr_{hp}_{m}")
                    nc.vector.tensor_tensor(prod[:], nrT[:, 0:4:2], rec[:], ALU.mult)
                    pv = fin.tile([128, 1], dt.float32, tag="pv", name=f"pv_{hp}_{m}")
                    nc.vector.tensor_reduce(pv[:], prod[:], axis=mybir.AxisListType.X, op=ALU.add)
                    nc.vector.tensor_tensor(v3acc[m][:], v3acc[m][:], pv[:], ALU.add)

            # ---- final delta + layernorm (stats from precomputed moments)
            for m in range(M4):
                v3 = fin.tile([128, 1], dt.float32, tag="v3", name=f"v3_{m}")
                nc.vector.tensor_scalar_add(v3[:], v3acc[m][:], cvec[:, 7:8])
                s3 = fin.tile([128, 1], dt.float32, tag="s3", name=f"s3_{m}")
                nc.vector.tensor_scalar_mul(s3[:], v3[:], a3s[m][:])
                nc.vector.tensor_tensor(s3[:], s3[:], b3s[m][:], ALU.subtract)
                # mu = (sum_x + s3*sum_k)/D
                mu = fin.tile([128, 1], dt.float32, tag="mu", name=f"mu_{m}")
                nc.vector.tensor_scalar_mul(mu[:], s3[:], mks3[m][:])
                nc.vector.tensor_tensor(mu[:], mu[:], mxs[m][:], ALU.add)
                nc.vector.tensor_scalar_mul(mu[:], mu[:], 1.0 / D)
                # E[y^2] = (xx + 2 s3 xk + s3^2 kk)/D ; var = E[y^2] - mu^2
                t1 = fin.tile([128, 1], dt.float32, tag="t1", name=f"t1_{m}")
                nc.vector.tensor_scalar_mul(t1[:], s3[:], kks3[m][:])
                t2 = fin.tile([128, 1], dt.float32, tag="t2", name=f"t2_{m}")
                nc.vector.tensor_scalar(t2[:], xks3[m][:], 2.0, None, ALU.mult)
                nc.vector.tensor_tensor(t2[:], t2[:], t1[:], ALU.add)
                nc.vector.tensor_scalar_mul(t2[:], t2[:], s3[:])
                nc.vector.tensor_tensor(t2[:], t2[:], xxs[m][:], ALU.add)
                var = fin.tile([128, 1], dt.float32, tag="var", name=f"var_{m}")
                nc.vector.tensor_scalar_mul(var[:], t2[:], 1.0 / D)
                mu2 = fin.tile([128, 1], dt.float32, tag="mu2", name=f"mu2_{m}")
                nc.vector.tensor_tensor(mu2[:], mu[:], mu[:], ALU.mult)
                nc.vector.tensor_tensor(var[:], var[:], mu2[:], ALU.subtract)
                nc.vector.tensor_scalar_add(var[:], var[:], LN_EPS)
                lnv2 = fin.tile([128, 1], dt.float32, tag="lnv2", name=f"lnv2_{m}")
                nc.scalar.activation(lnv2[:], var[:], AF.Ln)
                rstd = fin.tile([128, 1], dt.float32, tag="rstd", name=f"rstd_{m}")
                nc.scalar.activation(rstd[:], lnv2[:], AF.Exp, scale=-0.5)
                s3r = fin.tile([128, 1], dt.float32, tag="s3r", name=f"s3r_{m}")
                nc.vector.tensor_scalar_mul(s3r[:], s3[:], rstd[:])
                # yn = (x - mu)*rstd + k3raw*(s3*rstd) ; out = yn*g + b
                yn = fin.tile([128, D], dt.float32, tag="yn", name=f"yn_{m}")
                nc.vector.tensor_scalar(yn[:], x32[m][:], mu[:], rstd[:], ALU.subtract, ALU.mult)
                nc.vector.scalar_tensor_tensor(
                    yn[:], k3raw[m][:], s3r[:], yn[:], ALU.mult, ALU.add
                )
                yg = fin.tile([128, D], dt.float32, tag="yg", name=f"yg_{m}")
                nc.vector.tensor_tensor(yg[:], yn[:], lng[:], ALU.mult)
                nc.vector.tensor_tensor(yg[:], yg[:], lnb[:], ALU.add)
                nc.sync.dma_start(y_t[ts(m, 128), :], yg[:])

    _split_multi_waits(nc)
    nc.finalize()
    return nc


def _host_prep(inputs):
    """Precompute augmented weights and constants; returns per-core in_maps."""
    f32 = np.float32
    x = np.asarray(inputs["x"], f32)
    Wq, bq = np.asarray(inputs["Wq"], f32), np.asarray(inputs["bq"], f32)
    Wk, bk = np.asarray(inputs["Wk"], f32), np.asarray(inputs["bk"], f32)
    Wv, bv = np.asarray(inputs["Wv"], f32), np.asarray(inputs["bv"], f32)
    Wo, bo = np.asarray(inputs["Wo"], f32), np.asarray(inputs["bo"], f32)
    dWk, dbw = np.asarray(inputs["dWk"], f32), np.asarray(inputs["dbw"], f32)
    dbb, dWv = np.asarray(inputs["dbb"], f32), np.asarray(inputs["dWv"], f32)
    dbv = np.asarray(inputs["dbv"], f32)
    ln_g, ln_b = np.asarray(inputs["ln_g"], f32), np.asarray(inputs["ln_b"], f32)

    w = Wo @ dWv[3]                                   # (D,)
    Wu = np.zeros((D, H), f32)
    for h in range(H):
        Wu[h * HD:(h + 1) * HD, h] = w[h * HD:(h + 1) * HD]
    Bu = dWk[2] @ Wu                                  # (D, H)

    vw = [Wq @ dWv[0], Wk @ dWv[1], Wv @ dWv[2]]
    vc = [float(bq @ dWv[0] + dbv[0]), float(bk @ dWv[1] + dbv[1]),
          float(bv @ dWv[2] + dbv[2])]
    c3 = float(bo @ dWv[3] + dbv[3])

    bf = ml_dtypes.bfloat16
    augs = [np.ascontiguousarray(dWk[i]).astype(bf) for i in range(4)]
    ex = np.zeros((D, W_EX), f32)
    for i in range(4):
        ex[:, EX_DBW[i]] = dbw[i]
    for i in range(3):
        ex[:, EX_VW[i]] = vw[i]
    ex[:, EX_A:EX_A + H] = Wu
    ex[:, EX_B:EX_B + H] = Bu
    ex = ex.astype(bf)

    cvec = np.zeros((128, 16), f32)
    for i in range(4):
        cvec[:, i] = -dbb[i]
    for i in range(3):
        cvec[:, 4 + i] = vc[i]
    cvec[:, 7] = c3

    lng = np.broadcast_to(ln_g[None, :], (128, D)).copy()
    lnb = np.broadcast_to(ln_b[None, :], (128, D)).copy()

    xf = x.reshape(B * S, D)
    in_maps = []
    for c in range(N_CORES):
        m = {
            "x": np.ascontiguousarray(xf[c * TOK:(c + 1) * TOK]),
            "ex": ex, "cvec": cvec, "lng": lng, "lnb": lnb,
        }
        for i in range(4):
            m[f"aug{i}"] = augs[i]
        in_maps.append(m)
    return in_maps


def kernel(**inputs):
    global LAST_RESULTS
    if "nc" not in _CACHE:
        _CACHE["nc"] = _build_program()
    nc = _CACHE["nc"]
    in_maps = _host_prep(inputs)
    res = run_bass_kernel_spmd(nc, in_maps, core_ids=list(range(N_CORES)))
    LAST_RESULTS = res
    out = np.concatenate(
        [res.results[c]["y"] for c in range(N_CORES)], axis=0
    ).reshape(B, S, D)
    return out.astype(np.float32)


# revision 28
# speedup vs baseline: 1.0805x; 1.0057x over previous
"""DeltaAttention Trainium2 kernel — 8-core SPMD via bass/Tile.

Math (per reference): 4 DeltaResidualBlocks (d_v=1) wrapped around MHA.
Because each delta block consumes its v_in only through the scalar
projection v_in @ dWv[i], the Wq/Wk/Wv/Wo matmuls collapse into single
extra columns of the dWk matmuls (precomputed on host), and attn@v
collapses to 2 output columns per head:
    n_h[q] = E_h[q,:] @ u_h,  r_h[q] = E_h[q,:] @ 1,  u_h = v_h @ w_h
    v3[q]  = sum_h n_h/r_h + const,   w = Wo @ dWv[3]
Sharding: 512 query tokens per core; k^T and u are AllGathered within
each 4-core batch group.  LayerNorm statistics are computed from
precomputed moments of x and k3_raw (y = x + s3*k3_raw).
"""

import os
from contextlib import ExitStack

import numpy as np
import ml_dtypes

import concourse.bass as bass
import concourse.mybir as mybir
import concourse.tile as tile
from concourse.bass_utils import run_bass_kernel_spmd
from concourse.masks import make_identity

dt = mybir.dt
AF = mybir.ActivationFunctionType
ALU = mybir.AluOpType
ts = bass.ts

N_CORES = 8
B, S, D, H = 2, 2048, 1024, 16
HD = D // H
TOK = (B * S) // N_CORES          # 512 query tokens per core
M4 = TOK // 128                   # 4 token chunks
K8 = D // 128                     # 8 feature chunks
NKC = S // 128                    # 16 key chunks per batch
EPS = 1e-8
LN_EPS = 1e-5

# extras matmul columns: [dbw0,vw0, dbw1,vw1, dbw2,vw2, Wu(16), Bu(16), dbw3]
W_EX = 39
EX_DBW = [0, 2, 4, 38]
EX_VW = [1, 3, 5]
EX_A = 6      # 6..22  = x @ Wu
EX_B = 22     # 22..38 = x @ dWk2 @ Wu

# dtype of q^T / k^T used by the scores matmul (fp8 halves the AllGather)
SC_DT = dt.float8e4 if os.environ.get("DELTA_SCORES_FP8", "1") == "1" else dt.bfloat16

LAST_RESULTS = None
_CACHE = {}


def _split_multi_waits(nc, max_waits=1):
    """walrus (CoreV3) only encodes one sync wait per instruction; Tile's
    final drain can carry several. Hoist extras onto preceding NoOps."""
    n_fixed = 0
    for f in nc.m.functions:
        for blk in f.blocks:
            new_insts = []
            for inst in blk.instructions:
                si = inst.sync_info
                waits = list(si.on_wait) if (si and si.on_wait) else []
                if len(waits) > max_waits:
                    head, tail = waits[:-max_waits], waits[-max_waits:]
                    for j, w in enumerate(head):
                        nop = mybir.InstNoOp(
                            name=f"{inst.name}_waitsplit_{j}",
                            engine=inst.engine,
                            ins=[],
                            outs=[],
                            sync_info=mybir.SyncInfo(on_wait=[w], on_update=[]),
                        )
                        nc.register_instruction(nop)
                        new_insts.append(nop)
                        n_fixed += 1
                    si.on_wait = tail
                new_insts.append(inst)
            blk.instructions[:] = new_insts
    return n_fixed


def _build_program():
    nc = bass.Bass(num_devices=N_CORES)

    x_t = nc.dram_tensor("x", [TOK, D], dt.float32, kind="ExternalInput")
    aug_t = [
        nc.dram_tensor(f"aug{i}", [D, D], dt.bfloat16, kind="ExternalInput")
        for i in range(4)
    ]
    ex_t = nc.dram_tensor("ex", [D, W_EX], dt.bfloat16, kind="ExternalInput")
    cvec_t = nc.dram_tensor("cvec", [128, 16], dt.float32, kind="ExternalInput")
    lng_t = nc.dram_tensor("lng", [128, D], dt.float32, kind="ExternalInput")
    lnb_t = nc.dram_tensor("lnb", [128, D], dt.float32, kind="ExternalInput")
    y_t = nc.dram_tensor("y", [TOK, D], dt.float32, kind="ExternalOutput")

    RG = [[0, 1, 2, 3], [4, 5, 6, 7]]

    with tile.TileContext(nc) as tc, ExitStack() as stack:
        const = stack.enter_context(tc.tile_pool(name="const", bufs=1))
        dram = stack.enter_context(tc.tile_pool(name="dram", bufs=1, space="DRAM"))
        big = stack.enter_context(tc.tile_pool(name="big", bufs=1))

        agk_in = dram.tile([D, TOK], SC_DT, tag="agk_in")
        agk_pc = [
            dram.tile([4 * 256, TOK], SC_DT, tag=f"agk_pc{j}", name=f"agk_pc{j}")
            for j in range(4)
        ]
        agu_in = dram.tile([TOK, H], dt.bfloat16, tag="agu_in")
        agu_out = dram.tile([4 * TOK, H], dt.bfloat16, tag="agu_out")

        ident_bf = const.tile([128, 128], dt.bfloat16, tag="ident_bf")
        make_identity(nc, ident_bf[:])
        ident_f32 = const.tile([128, 128], dt.float32, tag="ident_f32")
        make_identity(nc, ident_f32[:])
        cvec = const.tile([128, 16], dt.float32, tag="cvec")
        nc.sync.dma_start(cvec[:], cvec_t[:])
        lng = const.tile([128, D], dt.float32, tag="lng")
        lnb = const.tile([128, D], dt.float32, tag="lnb")
        nc.sync.dma_start(lng[:], lng_t[:])
        nc.sync.dma_start(lnb[:], lnb_t[:])

        # persistent data tiles
        x32 = [big.tile([128, D], dt.float32, tag=f"x32_{m}", name=f"x32_{m}") for m in range(M4)]
        xbf = [big.tile([128, D], dt.bfloat16, tag=f"xbf_{m}", name=f"xbf_{m}") for m in range(M4)]
        xT = [big.tile([128, TOK], dt.bfloat16, tag=f"xT_{k}", name=f"xT_{k}") for k in range(K8)]
        qT = [big.tile([128, TOK], SC_DT, tag=f"qT_{k}", name=f"qT_{k}") for k in range(K8)]
        k3raw = [big.tile([128, D], dt.bfloat16, tag=f"k3_{m}", name=f"k3_{m}") for m in range(M4)]
        a3s = [big.tile([128, 1], dt.float32, tag=f"a3_{m}", name=f"a3_{m}") for m in range(M4)]
        b3s = [big.tile([128, 1], dt.float32, tag=f"b3_{m}", name=f"b3_{m}") for m in range(M4)]
        u_bf = [big.tile([128, H], dt.bfloat16, tag=f"u_{m}", name=f"u_{m}") for m in range(M4)]
        exsb = [big.tile([128, W_EX], dt.float32, tag=f"ex_{m}", name=f"ex_{m}") for m in range(M4)]
        v3acc = [big.tile([128, 1], dt.float32, tag=f"v3a_{m}", name=f"v3a_{m}") for m in range(M4)]
        mxs = [big.tile([128, 1], dt.float32, tag=f"mx_{m}", name=f"mx_{m}") for m in range(M4)]
        xxs = [big.tile([128, 1], dt.float32, tag=f"xx_{m}", name=f"xx_{m}") for m in range(M4)]
        mks3 = [big.tile([128, 1], dt.float32, tag=f"mk3_{m}", name=f"mk3_{m}") for m in range(M4)]
        kks3 = [big.tile([128, 1], dt.float32, tag=f"kk3_{m}", name=f"kk3_{m}") for m in range(M4)]
        xks3 = [big.tile([128, 1], dt.float32, tag=f"xk3_{m}", name=f"xk3_{m}") for m in range(M4)]
        nrw = big.tile([2, 2 * TOK], dt.float32, tag="nrw")
        aug3t = [
            big.tile([128, D], dt.bfloat16, tag=f"aug3_{k}", name=f"aug3w_{k}")
            for k in range(K8)
        ]

        for m in range(M4):
            nc.sync.dma_start(x32[m][:], x_t[ts(m, 128), :])
            nc.scalar.copy(xbf[m][:], x32[m][:])
            nc.vector.memset(v3acc[m][:], 0.0)
            nc.vector.tensor_reduce(mxs[m][:], x32[m][:], axis=mybir.AxisListType.X, op=ALU.add)
        for k in range(K8):
            nc.sync.dma_start(aug3t[k][:], aug_t[3][ts(k, 128), :])

        with (
            tc.tile_pool(name="wpool", bufs=16) as wpool,
            tc.tile_pool(name="qkpool", bufs=4) as qkpool,
            tc.tile_pool(name="scpool", bufs=24) as scpool,
            tc.tile_pool(name="scr", bufs=2) as scrpool,
            tc.tile_pool(name="ktloc", bufs=8) as ktlpool,
            tc.tile_pool(name="pp_proj", bufs=2, space="PSUM") as pp_proj,
            tc.tile_pool(name="pp_ex", bufs=2, space="PSUM") as pp_ex,
            tc.tile_pool(name="pp_t", bufs=2, space="PSUM") as pp_t,
        ):
            # x^T via PE transpose (bf16)
            for k in range(K8):
                pst = pp_t.tile([128, TOK], dt.bfloat16, tag="pst")
                for m in range(M4):
                    nc.tensor.transpose(
                        pst[:, ts(m, 128)], xbf[m][:, ts(k, 128)], ident_bf[:]
                    )
                nc.vector.tensor_copy(xT[k][:], pst[:])

            # extras matmul: all betas / v-scalars / u components at once
            ext = [wpool.tile([128, W_EX], dt.bfloat16, tag="ext", name=f"ext_{k}") for k in range(K8)]
            for k in range(K8):
                nc.sync.dma_start(ext[k][:], ex_t[ts(k, 128), :])
            for m in range(M4):
                pse = pp_ex.tile([128, W_EX], dt.float32, tag="pse")
                for k in range(K8):
                    nc.tensor.matmul(
                        pse[:], xT[k][:, ts(m, 128)], ext[k][:],
                        start=(k == 0), stop=(k == K8 - 1),
                    )
                nc.vector.tensor_copy(exsb[m][:], pse[:])

            qk_out = {}

            def scalar_chain(i, m, ps_beta_src, kx, rnorm):
                """beta, rk, rr from per-chunk scalars. Returns (rk, rr)."""
                ez = scpool.tile([128, 1], dt.float32, tag="sc", name=f"ez_{i}_{m}")
                nc.scalar.activation(
                    ez[:], ps_beta_src, AF.Exp, scale=-1.0, bias=cvec[:, i:i + 1]
                )
                ez1 = scpool.tile([128, 1], dt.float32, tag="sc", name=f"ez1_{i}_{m}")
                nc.vector.tensor_scalar_add(ez1[:], ez[:], 1.0)
                rsig = scpool.tile([128, 1], dt.float32, tag="sc", name=f"rs_{i}_{m}")
                nc.vector.reciprocal(rsig[:], ez1[:])
                rk = scpool.tile([128, 1], dt.float32, tag="sc", name=f"rk_{i}_{m}")
                nc.vector.tensor_scalar_mul(rk[:], kx[:], rnorm[:])
                rr = scpool.tile([128, 1], dt.float32, tag="sc", name=f"rr_{i}_{m}")
                nc.vector.tensor_scalar(rr[:], rsig[:], rnorm[:], 2.0, ALU.mult, ALU.mult)
                return rk, rr

            def rnorm_chain(i, m, ss):
                lnv = scpool.tile([128, 1], dt.float32, tag="sc", name=f"lnv_{i}_{m}")
                nc.scalar.activation(lnv[:], ss[:], AF.Ln)
                nrm = scpool.tile([128, 1], dt.float32, tag="sc", name=f"nrm_{i}_{m}")
                nc.scalar.activation(nrm[:], lnv[:], AF.Exp, scale=0.5)
                nrme = scpool.tile([128, 1], dt.float32, tag="sc", name=f"nrme_{i}_{m}")
                nc.vector.tensor_scalar_add(nrme[:], nrm[:], EPS)
                rnorm = scpool.tile([128, 1], dt.float32, tag="sc", name=f"rn_{i}_{m}")
                nc.vector.reciprocal(rnorm[:], nrme[:])
                return rnorm

            def delta_block(i):
                """dWk matmul + delta elementwise for aug i on all 4 chunks."""
                augt = [
                    wpool.tile([128, D], dt.bfloat16, tag="aug", name=f"aug_{i}_{k}")
                    for k in range(K8)
                ]
                for k in range(K8):
                    nc.sync.dma_start(augt[k][:], aug_t[i][ts(k, 128), :])
                outs = []
                for m in range(M4):
                    ps = pp_proj.tile([128, D], dt.float32, tag="ps_proj")
                    for k in range(K8):
                        for s0 in (0, 512):
                            nc.tensor.matmul(
                                ps[:, s0:s0 + 512], xT[k][:, ts(m, 128)],
                                augt[k][:, s0:s0 + 512],
                                start=(k == 0), stop=(k == K8 - 1),
                            )
                    ex = exsb[m]
                    scr = scrpool.tile([128, D], dt.bfloat16, tag="scr", name=f"scr_{i}_{m}")
                    ss = scpool.tile([128, 1], dt.float32, tag="sc", name=f"ss_{i}_{m}")
                    nc.scalar.activation(scr[:], ps[:], AF.Square, accum_out=ss[:])
                    kx = scpool.tile([128, 1], dt.float32, tag="sc", name=f"kx_{i}_{m}")
                    scr2 = scrpool.tile([128, D], dt.bfloat16, tag="scr", name=f"scr2_{i}_{m}")
                    nc.vector.scalar_tensor_tensor(
                        scr2[:], ps[:], 1.0, x32[m][:], ALU.mult, ALU.mult,
                        accum_out=kx[:],
                    )
                    rnorm = rnorm_chain(i, m, ss)
                    rk, rr = scalar_chain(i, m, ex[:, EX_DBW[i]:EX_DBW[i] + 1], kx, rnorm)
                    v = scpool.tile([128, 1], dt.float32, tag="sc", name=f"v_{i}_{m}")
                    nc.vector.tensor_scalar_add(
                        v[:], ex[:, EX_VW[i]:EX_VW[i] + 1], cvec[:, 4 + i:5 + i]
                    )
                    dv = scpool.tile([128, 1], dt.float32, tag="sc", name=f"dv_{i}_{m}")
                    nc.vector.tensor_tensor(dv[:], v[:], rk[:], ALU.subtract)
                    s = scpool.tile([128, 1], dt.float32, tag="sc", name=f"s_{i}_{m}")
                    nc.vector.tensor_tensor(s[:], dv[:], rr[:], ALU.mult)
                    if i in (0, 1):
                        o = qkpool.tile([128, D], dt.bfloat16, tag="qk", name=f"qk_{i}_{m}")
                        nc.vector.scalar_tensor_tensor(
                            o[:], ps[:], s[:], x32[m][:], ALU.mult, ALU.add
                        )
                        outs.append(o)
                    else:
                        # i == 2: u = A + s*B  (A/B live in the extras tile)
                        nc.vector.scalar_tensor_tensor(
                            u_bf[m][:], ex[:, EX_B:EX_B + H], s[:], ex[:, EX_A:EX_A + H],
                            ALU.mult, ALU.add,
                        )
                qk_out[i] = outs

            def delta3_chunk(m):
                """dWk3 matmul; elementwise on DVE from SBUF copy; LN moments."""
                psd = pp_proj.tile([128, D], dt.float32, tag="ps_proj")
                for k in range(K8):
                    for s0 in (0, 512):
                        nc.tensor.matmul(
                            psd[:, s0:s0 + 512], xT[k][:, ts(m, 128)],
                            aug3t[k][:, s0:s0 + 512],
                            start=(k == 0), stop=(k == K8 - 1),
                        )
                mka = scpool.tile([128, 1], dt.float32, tag="sc", name=f"mka_{m}")
                mkb = scpool.tile([128, 1], dt.float32, tag="sc", name=f"mkb_{m}")
                nc.vector.tensor_scalar(
                    k3raw[m][:, 0:512], psd[:, 0:512], 1.0, 0.0, ALU.mult,
                    ALU.add, accum_out=mka[:],
                )
                nc.vector.tensor_scalar(
                    k3raw[m][:, 512:1024], psd[:, 512:1024], 1.0, 0.0, ALU.mult,
                    ALU.add, accum_out=mkb[:],
                )
                nc.vector.tensor_tensor(mks3[m][:], mka[:], mkb[:], ALU.add)
                scr = scrpool.tile([128, D], dt.bfloat16, tag="scr", name=f"sc3r_{m}")
                nc.vector.scalar_tensor_tensor(
                    scr[:], k3raw[m][:], 1.0, k3raw[m][:], ALU.mult, ALU.mult,
                    accum_out=kks3[m][:],
                )
                scr2 = scrpool.tile([128, D], dt.bfloat16, tag="scr", name=f"sc3r2_{m}")
                nc.vector.scalar_tensor_tensor(
                    scr2[:], k3raw[m][:], 1.0, xbf[m][:], ALU.mult, ALU.mult,
                    accum_out=xks3[m][:],
                )
                rnorm = rnorm_chain(3, m, kks3[m])
                rk, rr = scalar_chain(3, m, exsb[m][:, EX_DBW[3]:EX_DBW[3] + 1], xks3[m], rnorm)
                nc.vector.tensor_copy(a3s[m][:], rr[:])
                nc.vector.tensor_tensor(b3s[m][:], rr[:], rk[:], ALU.mult)

            def transpose_to(src_tiles, dst_tiles):
                for k in range(K8):
                    pst = pp_t.tile([128, TOK], dt.bfloat16, tag="pst")
                    for m in range(M4):
                        nc.tensor.transpose(
                            pst[:, ts(m, 128)], src_tiles[m][:, ts(k, 128)], ident_bf[:]
                        )
                    nc.vector.tensor_copy(dst_tiles[k][:], pst[:])

            # ---- k path first so the AllGather starts early
            delta_block(1)
            ktloc = [ktlpool.tile([128, TOK], SC_DT, tag="ktloc", name=f"ktloc_{k}") for k in range(K8)]
            transpose_to(qk_out[1], ktloc)
            for k in range(K8):
                nc.sync.dma_start(agk_in[ts(k, 128), :], ktloc[k][:])
            nc.gpsimd.collective_compute(
                "AllGather", ALU.bypass, ins=[agk_in[0:256, :]], outs=[agk_pc[0][:]],
                replica_groups=RG,
            )
            delta_block(2)
            for m in range(M4):
                nc.sync.dma_start(agu_in[ts(m, 128), :], u_bf[m][:])
            nc.gpsimd.collective_compute(
                "AllGather", ALU.bypass, ins=[agu_in[:]], outs=[agu_out[:]],
                replica_groups=RG,
            )
            for j in range(1, 4):
                nc.gpsimd.collective_compute(
                    "AllGather", ALU.bypass,
                    ins=[agk_in[256 * j:256 * (j + 1), :]], outs=[agk_pc[j][:]],
                    replica_groups=RG,
                )
            delta_block(0)
            transpose_to(qk_out[0], qT)
            for m in range(M4):
                delta3_chunk(m)
                xsq = scrpool.tile([128, D], dt.bfloat16, tag="scr", name=f"xsq_{m}")
                nc.scalar.activation(xsq[:], x32[m][:], AF.Square, accum_out=xxs[m][:])

        # ---------------- attention ----------------
        with (
            tc.tile_pool(name="attn_sb", bufs=1) as attn_sb,
            tc.tile_pool(name="epool", bufs=4) as epool,
            tc.tile_pool(name="fin", bufs=2) as fin,
            tc.tile_pool(name="pp_sc", bufs=3, space="PSUM") as pp_sc,
            tc.tile_pool(name="pp_nr", bufs=2, space="PSUM") as pp_nr,
        ):
            kT = [attn_sb.tile([128, S], SC_DT, tag=f"kT_{k}", name=f"kTsb_{k}") for k in range(K8)]
            for k in range(K8):
                src = agk_pc[k // 2][:].rearrange("(c d) t -> d c t", c=4)[ts(k % 2, 128), :, :]
                dst = kT[k][:].rearrange("p (c t) -> p c t", c=4)
                nc.sync.dma_start(dst, src)
            uext = attn_sb.tile([128, NKC, H, 2], dt.bfloat16, tag="uext")
            nc.vector.memset(uext[:], 1.0)
            u_all = attn_sb.tile([128, NKC, H], dt.bfloat16, tag="u_all")
            nc.sync.dma_start(
                u_all[:], agu_out[:].rearrange("(kc p) h -> p kc h", p=128)
            )
            nc.vector.tensor_copy(uext[:, :, :, 0], u_all[:])

            SCALE = float(HD) ** -0.5

            for hp in range(K8):         # 8 head pairs; pair hp = heads 2hp, 2hp+1
                nr_ps = pp_nr.tile([128, TOK], dt.float32, tag="nr")
                hA, hB = 2 * hp, 2 * hp + 1
                for kc in range(NKC):
                    ps2 = pp_sc.tile([128, 2, TOK], dt.float32, tag="sc2")
                    nc.tensor.matmul(
                        ps2[:, 0, :], kT[hp][0:64, ts(kc, 128)], qT[hp][0:64, :],
                        start=True, stop=True, tile_position=(0, 0),
                    )
                    nc.tensor.matmul(
                        ps2[:, 1, :], kT[hp][64:128, ts(kc, 128)], qT[hp][64:128, :],
                        start=True, stop=True, tile_position=(64, 0),
                    )
                    E = epool.tile([128, 2, TOK], dt.bfloat16, tag="E")
                    nc.scalar.activation(E[:], ps2[:], AF.Exp, scale=SCALE)
                    nc.tensor.matmul(
                        nr_ps[0:2, :], uext[:, kc, hA, :], E[:, 0, :],
                        start=(kc == 0), stop=(kc == NKC - 1),
                        tile_position=(0, 0),
                    )
                    nc.tensor.matmul(
                        nr_ps[32:34, :], uext[:, kc, hB, :], E[:, 1, :],
                        start=(kc == 0), stop=(kc == NKC - 1),
                        tile_position=(0, 32),
                    )
                # stage the pair's n/r rows and fold into v3acc (overlaps attention)
                for j in range(2):
                    nc.vector.tensor_copy(
                        nrw[0:2, j * TOK:(j + 1) * TOK], nr_ps[32 * j:32 * j + 2, :]
                    )
                for m in range(M4):
                    psT = pp_nr.tile([128, TOK], dt.float32, tag="nr")
                    for j in range(2):
                        nc.tensor.transpose(
                            psT[:, 2 * j:2 * j + 2],
                            nrw[0:2, j * TOK + 128 * m: j * TOK + 128 * (m + 1)],
                            ident_f32[0:2, 0:2],
                        )
                    nrT = fin.tile([128, 4], dt.float32, tag="nrT", name=f"nrT_{hp}_{m}")
                    nc.vector.tensor_copy(nrT[:], psT[:, 0:4])
                    rec = fin.tile([128, 2], dt.float32, tag="rec", name=f"rec_{hp}_{m}")
                    nc.vector.reciprocal(rec[:], nrT[:, 1:4:2])
                    prod = fin.tile([128, 2], dt.float32, tag="prod", name=f"pr_{hp}_{m}")
                    nc.vector.tensor_tensor(prod[:], nrT[:, 0:4:2], rec[:], ALU.mult)
                    pv = fin.tile([128, 1], dt.float32, tag="pv", name=f"pv_{hp}_{m}")
                    nc.vector.tensor_reduce(pv[:], prod[:], axis=mybir.AxisListType.X, op=ALU.add)
                    nc.vector.tensor_tensor(v3acc[m][:], v3acc[m][:], pv[:], ALU.add)

            # ---- final delta + layernorm (stats from precomputed moments)
            for m in range(M4):
                v3 = fin.tile([128, 1], dt.float32, tag="v3", name=f"v3_{m}")
                nc.vector.tensor_scalar_add(v3[:], v3acc[m][:], cvec[:, 7:8])
                s3 = fin.tile([128, 1], dt.float32, tag="s3", name=f"s3_{m}")
                nc.vector.tensor_scalar_mul(s3[:], v3[:], a3s[m][:])
                nc.vector.tensor_tensor(s3[:], s3[:], b3s[m][:], ALU.subtract)
                # mu = (sum_x + s3*sum_k)/D
                mu = fin.tile([128, 1], dt.float32, tag="mu", name=f"mu_{m}")
                nc.vector.tensor_scalar_mul(mu[:], s3[:], mks3[m][:])
                nc.vector.tensor_tensor(mu[:], mu[:], mxs[m][:], ALU.add)
                nc.vector.tensor_scalar_mul(mu[:], mu[:], 1.0 / D)
                # E[y^2] = (xx + 2 s3 xk + s3^2 kk)/D ; var = E[y^2] - mu^2
                t1 = fin.tile([128, 1], dt.float32, tag="t1", name=f"t1_{m}")
                nc.vector.tensor_scalar_mul(t1[:], s3[:], kks3[m][:])
                t2 = fin.tile([128, 1], dt.float32, tag="t2", name=f"t2_{m}")
                nc.vector.tensor_scalar(t2[:], xks3[m][:], 2.0, None, ALU.mult)
                nc.vector.tensor_tensor(t2[:], t2[:], t1[:], ALU.add)
                nc.vector.tensor_scalar_mul(t2[:], t2[:], s3[:])
                nc.vector.tensor_tensor(t2[:], t2[:], xxs[m][:], ALU.add)
                var = fin.tile([128, 1], dt.float32, tag="var", name=f"var_{m}")
                nc.vector.tensor_scalar_mul(var[:], t2[:], 1.0 / D)
                mu2 = fin.tile([128, 1], dt.float32, tag="mu2", name=f"mu2_{m}")
                nc.vector.tensor_tensor(mu2[:], mu[:], mu[:], ALU.mult)
                nc.vector.tensor_tensor(var[:], var[:], mu2[:], ALU.subtract)
                nc.vector.tensor_scalar_add(var[:], var[:], LN_EPS)
                lnv2 = fin.tile([128, 1], dt.float32, tag="lnv2", name=f"lnv2_{m}")
                nc.scalar.activation(lnv2[:], var[:], AF.Ln)
                rstd = fin.tile([128, 1], dt.float32, tag="rstd", name=f"rstd_{m}")
                nc.scalar.activation(rstd[:], lnv2[:], AF.Exp, scale=-0.5)
                s3r = fin.tile([128, 1], dt.float32, tag="s3r", name=f"s3r_{m}")
                nc.vector.tensor_scalar_mul(s3r[:], s3[:], rstd[:])
                # yn = (x - mu)*rstd + k3raw*(s3*rstd) ; out = yn*g + b
                yn = fin.tile([128, D], dt.float32, tag="yn", name=f"yn_{m}")
                nc.vector.tensor_scalar(yn[:], x32[m][:], mu[:], rstd[:], ALU.subtract, ALU.mult)
                nc.vector.scalar_tensor_tensor(
                    yn[:], k3raw[m][:], s3r[:], yn[:], ALU.mult, ALU.add
                )
                yg = fin.tile([128, D], dt.float32, tag="yg", name=f"yg_{m}")
                eng = nc.vector if m % 2 == 0 else nc.gpsimd
                eng.tensor_tensor(yg[:], yn[:], lng[:], ALU.mult)
                eng.tensor_tensor(yg[:], yg[:], lnb[:], ALU.add)
                nc.sync.dma_start(y_t[ts(m, 128), :], yg[:])

    _split_multi_waits(nc)
    nc.finalize()
    return nc


def _host_prep(inputs):
    """Precompute augmented weights and constants; returns per-core in_maps."""
    f32 = np.float32
    x = np.asarray(inputs["x"], f32)
    Wq, bq = np.asarray(inputs["Wq"], f32), np.asarray(inputs["bq"], f32)
    Wk, bk = np.asarray(inputs["Wk"], f32), np.asarray(inputs["bk"], f32)
    Wv, bv = np.asarray(inputs["Wv"], f32), np.asarray(inputs["bv"], f32)
    Wo, bo = np.asarray(inputs["Wo"], f32), np.asarray(inputs["bo"], f32)
    dWk, dbw = np.asarray(inputs["dWk"], f32), np.asarray(inputs["dbw"], f32)
    dbb, dWv = np.asarray(inputs["dbb"], f32), np.asarray(inputs["dWv"], f32)
    dbv = np.asarray(inputs["dbv"], f32)
    ln_g, ln_b = np.asarray(inputs["ln_g"], f32), np.asarray(inputs["ln_b"], f32)

    w = Wo @ dWv[3]                                   # (D,)
    Wu = np.zeros((D, H), f32)
    for h in range(H):
        Wu[h * HD:(h + 1) * HD, h] = w[h * HD:(h + 1) * HD]
    Bu = dWk[2] @ Wu                                  # (D, H)

    vw = [Wq @ dWv[0], Wk @ dWv[1], Wv @ dWv[2]]
    vc = [float(bq @ dWv[0] + dbv[0]), float(bk @ dWv[1] + dbv[1]),
          float(bv @ dWv[2] + dbv[2])]
    c3 = float(bo @ dWv[3] + dbv[3])

    bf = ml_dtypes.bfloat16
    augs = [np.ascontiguousarray(dWk[i]).astype(bf) for i in range(4)]
    ex = np.zeros((D, W_EX), f32)
    for i in range(4):
        ex[:, EX_DBW[i]] = dbw[i]
    for i in range(3):
        ex[:, EX_VW[i]] = vw[i]
    ex[:, EX_A:EX_A + H] = Wu
    ex[:, EX_B:EX_B + H] = Bu
    ex = ex.astype(bf)

    cvec = np.zeros((128, 16), f32)
    for i in range(4):
        cvec[:, i] = -dbb[i]
    for i in range(3):
        cvec[:, 4 + i] = vc[i]
    cvec[:, 7] = c3

    lng = np.broadcast_to(ln_g[None, :], (128, D)).copy()
    lnb = np.broadcast_to(ln_b[None, :], (128, D)).copy()

    xf = x.reshape(B * S, D)
    in_maps = []
    for c in range(N_CORES):
        m = {
            "x": np.ascontiguousarray(xf[c * TOK:(c + 1) * TOK]),
            "ex": ex, "cvec": cvec, "lng": lng, "lnb": lnb,
        }
        for i in range(4):
            m[f"aug{i}"] = augs[i]
        in_maps.append(m)
    return in_maps


def kernel(**inputs):
    global LAST_RESULTS
    if "nc" not in _CACHE:
        _CACHE["nc"] = _build_program()
    nc = _CACHE["nc"]
    in_maps = _host_prep(inputs)
    res = run_bass_kernel_spmd(nc, in_maps, core_ids=list(range(N_CORES)))
    LAST_RESULTS = res
    out = np.concatenate(
        [res.results[c]["y"] for c in range(N_CORES)], axis=0
    ).reshape(B, S, D)
    return out.astype(np.float32)


# revision 29
# speedup vs baseline: 1.1386x; 1.0537x over previous
"""DeltaAttention Trainium2 kernel — 8-core SPMD via bass/Tile.

Math (per reference): 4 DeltaResidualBlocks (d_v=1) wrapped around MHA.
Because each delta block consumes its v_in only through the scalar
projection v_in @ dWv[i], the Wq/Wk/Wv/Wo matmuls collapse into single
extra columns of the dWk matmuls (precomputed on host), and attn@v
collapses to 2 output columns per head:
    n_h[q] = E_h[q,:] @ u_h,  r_h[q] = E_h[q,:] @ 1,  u_h = v_h @ w_h
    v3[q]  = sum_h n_h/r_h + const,   w = Wo @ dWv[3]
Sharding: 512 query tokens per core; k^T and u are AllGathered within
each 4-core batch group.  LayerNorm statistics are computed from
precomputed moments of x and k3_raw (y = x + s3*k3_raw).
"""

import os
from contextlib import ExitStack

import numpy as np
import ml_dtypes

import concourse.bass as bass
import concourse.mybir as mybir
import concourse.tile as tile
from concourse.bass_utils import run_bass_kernel_spmd
from concourse.masks import make_identity

dt = mybir.dt
AF = mybir.ActivationFunctionType
ALU = mybir.AluOpType
ts = bass.ts

N_CORES = 8
B, S, D, H = 2, 2048, 1024, 16
HD = D // H
TOK = (B * S) // N_CORES          # 512 query tokens per core
M4 = TOK // 128                   # 4 token chunks
K8 = D // 128                     # 8 feature chunks
NKC = S // 128                    # 16 key chunks per batch
EPS = 1e-8
LN_EPS = 1e-5

# extras matmul columns: [dbw0,vw0, dbw1,vw1, dbw2,vw2, Wu(16), Bu(16), dbw3]
W_EX = 39
EX_DBW = [0, 2, 4, 38]
EX_VW = [1, 3, 5]
EX_A = 6      # 6..22  = x @ Wu
EX_B = 22     # 22..38 = x @ dWk2 @ Wu

# dtype of q^T / k^T used by the scores matmul (fp8 halves the AllGather)
SC_DT = dt.float8e4 if os.environ.get("DELTA_SCORES_FP8", "1") == "1" else dt.bfloat16

LAST_RESULTS = None
_CACHE = {}


def _split_multi_waits(nc, max_waits=1):
    """walrus (CoreV3) only encodes one sync wait per instruction; Tile's
    final drain can carry several. Hoist extras onto preceding NoOps."""
    n_fixed = 0
    for f in nc.m.functions:
        for blk in f.blocks:
            new_insts = []
            for inst in blk.instructions:
                si = inst.sync_info
                waits = list(si.on_wait) if (si and si.on_wait) else []
                if len(waits) > max_waits:
                    head, tail = waits[:-max_waits], waits[-max_waits:]
                    for j, w in enumerate(head):
                        nop = mybir.InstNoOp(
                            name=f"{inst.name}_waitsplit_{j}",
                            engine=inst.engine,
                            ins=[],
                            outs=[],
                            sync_info=mybir.SyncInfo(on_wait=[w], on_update=[]),
                        )
                        nc.register_instruction(nop)
                        new_insts.append(nop)
                        n_fixed += 1
                    si.on_wait = tail
                new_insts.append(inst)
            blk.instructions[:] = new_insts
    return n_fixed


def _build_program():
    nc = bass.Bass(num_devices=N_CORES)

    x_t = nc.dram_tensor("x", [TOK, D], dt.float32, kind="ExternalInput")
    aug_t = [
        nc.dram_tensor(f"aug{i}", [D, D], dt.bfloat16, kind="ExternalInput")
        for i in range(4)
    ]
    ex_t = nc.dram_tensor("ex", [D, W_EX], dt.bfloat16, kind="ExternalInput")
    cvec_t = nc.dram_tensor("cvec", [128, 16], dt.float32, kind="ExternalInput")
    lng_t = nc.dram_tensor("lng", [128, D], dt.float32, kind="ExternalInput")
    lnb_t = nc.dram_tensor("lnb", [128, D], dt.float32, kind="ExternalInput")
    y_t = nc.dram_tensor("y", [TOK, D], dt.float32, kind="ExternalOutput")

    RG = [[0, 1, 2, 3], [4, 5, 6, 7]]

    with tile.TileContext(nc) as tc, ExitStack() as stack:
        const = stack.enter_context(tc.tile_pool(name="const", bufs=1))
        dram = stack.enter_context(tc.tile_pool(name="dram", bufs=1, space="DRAM"))
        big = stack.enter_context(tc.tile_pool(name="big", bufs=1))

        agk_in = dram.tile([D, TOK], SC_DT, tag="agk_in")
        agk_pc = [
            dram.tile([4 * 256, TOK], SC_DT, tag=f"agk_pc{j}", name=f"agk_pc{j}")
            for j in range(4)
        ]
        agu_in = dram.tile([TOK, H], dt.bfloat16, tag="agu_in")
        agu_out = dram.tile([4 * TOK, H], dt.bfloat16, tag="agu_out")

        ident_bf = const.tile([128, 128], dt.bfloat16, tag="ident_bf")
        make_identity(nc, ident_bf[:])
        ident_f32 = const.tile([128, 128], dt.float32, tag="ident_f32")
        make_identity(nc, ident_f32[:])
        cvec = const.tile([128, 16], dt.float32, tag="cvec")
        nc.sync.dma_start(cvec[:], cvec_t[:])
        lng = const.tile([128, D], dt.float32, tag="lng")
        lnb = const.tile([128, D], dt.float32, tag="lnb")
        nc.sync.dma_start(lng[:], lng_t[:])
        nc.sync.dma_start(lnb[:], lnb_t[:])

        # persistent data tiles
        x32 = [big.tile([128, D], dt.float32, tag=f"x32_{m}", name=f"x32_{m}") for m in range(M4)]
        xbf = [big.tile([128, D], dt.bfloat16, tag=f"xbf_{m}", name=f"xbf_{m}") for m in range(M4)]
        xT = [big.tile([128, TOK], dt.bfloat16, tag=f"xT_{k}", name=f"xT_{k}") for k in range(K8)]
        qT = [big.tile([128, TOK], SC_DT, tag=f"qT_{k}", name=f"qT_{k}") for k in range(K8)]
        k3raw = [big.tile([128, D], dt.bfloat16, tag=f"k3_{m}", name=f"k3_{m}") for m in range(M4)]
        a3s = [big.tile([128, 1], dt.float32, tag=f"a3_{m}", name=f"a3_{m}") for m in range(M4)]
        b3s = [big.tile([128, 1], dt.float32, tag=f"b3_{m}", name=f"b3_{m}") for m in range(M4)]
        u_bf = [big.tile([128, H], dt.bfloat16, tag=f"u_{m}", name=f"u_{m}") for m in range(M4)]
        exsb = [big.tile([128, W_EX], dt.float32, tag=f"ex_{m}", name=f"ex_{m}") for m in range(M4)]
        v3acc = [big.tile([128, 1], dt.float32, tag=f"v3a_{m}", name=f"v3a_{m}") for m in range(M4)]
        mxs = [big.tile([128, 1], dt.float32, tag=f"mx_{m}", name=f"mx_{m}") for m in range(M4)]
        xxs = [big.tile([128, 1], dt.float32, tag=f"xx_{m}", name=f"xx_{m}") for m in range(M4)]
        mks3 = [big.tile([128, 1], dt.float32, tag=f"mk3_{m}", name=f"mk3_{m}") for m in range(M4)]
        kks3 = [big.tile([128, 1], dt.float32, tag=f"kk3_{m}", name=f"kk3_{m}") for m in range(M4)]
        xks3 = [big.tile([128, 1], dt.float32, tag=f"xk3_{m}", name=f"xk3_{m}") for m in range(M4)]
        nrw = big.tile([2, 2 * TOK], dt.float32, tag="nrw")
        aug3t = [
            big.tile([128, D], dt.bfloat16, tag=f"aug3_{k}", name=f"aug3w_{k}")
            for k in range(K8)
        ]

        for m in range(M4):
            nc.sync.dma_start(x32[m][:], x_t[ts(m, 128), :])
            nc.scalar.copy(xbf[m][:], x32[m][:])
            nc.vector.memset(v3acc[m][:], 0.0)
            nc.vector.tensor_reduce(mxs[m][:], x32[m][:], axis=mybir.AxisListType.X, op=ALU.add)
        for k in range(K8):
            nc.sync.dma_start(aug3t[k][:], aug_t[3][ts(k, 128), :])

        with (
            tc.tile_pool(name="wpool", bufs=16) as wpool,
            tc.tile_pool(name="qkpool", bufs=4) as qkpool,
            tc.tile_pool(name="scpool", bufs=24) as scpool,
            tc.tile_pool(name="scr", bufs=2) as scrpool,
            tc.tile_pool(name="ktloc", bufs=8) as ktlpool,
            tc.tile_pool(name="pp_proj", bufs=2, space="PSUM") as pp_proj,
            tc.tile_pool(name="pp_ex", bufs=2, space="PSUM") as pp_ex,
            tc.tile_pool(name="pp_t", bufs=2, space="PSUM") as pp_t,
        ):
            # x^T via PE transpose (bf16)
            for k in range(K8):
                pst = pp_t.tile([128, TOK], dt.bfloat16, tag="pst")
                for m in range(M4):
                    nc.tensor.transpose(
                        pst[:, ts(m, 128)], xbf[m][:, ts(k, 128)], ident_bf[:]
                    )
                nc.vector.tensor_copy(xT[k][:], pst[:])

            # extras matmul: all betas / v-scalars / u components at once
            ext = [wpool.tile([128, W_EX], dt.bfloat16, tag="ext", name=f"ext_{k}") for k in range(K8)]
            for k in range(K8):
                nc.sync.dma_start(ext[k][:], ex_t[ts(k, 128), :])
            for m in range(M4):
                pse = pp_ex.tile([128, W_EX], dt.float32, tag="pse")
                for k in range(K8):
                    nc.tensor.matmul(
                        pse[:], xT[k][:, ts(m, 128)], ext[k][:],
                        start=(k == 0), stop=(k == K8 - 1),
                    )
                nc.vector.tensor_copy(exsb[m][:], pse[:])

            qk_out = {}

            def scalar_chain(i, m, ps_beta_src, kx, rnorm):
                """beta, rk, rr from per-chunk scalars. Returns (rk, rr)."""
                ez = scpool.tile([128, 1], dt.float32, tag="sc", name=f"ez_{i}_{m}")
                nc.scalar.activation(
                    ez[:], ps_beta_src, AF.Exp, scale=-1.0, bias=cvec[:, i:i + 1]
                )
                ez1 = scpool.tile([128, 1], dt.float32, tag="sc", name=f"ez1_{i}_{m}")
                nc.vector.tensor_scalar_add(ez1[:], ez[:], 1.0)
                rsig = scpool.tile([128, 1], dt.float32, tag="sc", name=f"rs_{i}_{m}")
                nc.vector.reciprocal(rsig[:], ez1[:])
                rk = scpool.tile([128, 1], dt.float32, tag="sc", name=f"rk_{i}_{m}")
                nc.vector.tensor_scalar_mul(rk[:], kx[:], rnorm[:])
                rr = scpool.tile([128, 1], dt.float32, tag="sc", name=f"rr_{i}_{m}")
                nc.vector.tensor_scalar(rr[:], rsig[:], rnorm[:], 2.0, ALU.mult, ALU.mult)
                return rk, rr

            def rnorm_chain(i, m, ss):
                lnv = scpool.tile([128, 1], dt.float32, tag="sc", name=f"lnv_{i}_{m}")
                nc.scalar.activation(lnv[:], ss[:], AF.Ln)
                nrm = scpool.tile([128, 1], dt.float32, tag="sc", name=f"nrm_{i}_{m}")
                nc.scalar.activation(nrm[:], lnv[:], AF.Exp, scale=0.5)
                nrme = scpool.tile([128, 1], dt.float32, tag="sc", name=f"nrme_{i}_{m}")
                nc.vector.tensor_scalar_add(nrme[:], nrm[:], EPS)
                rnorm = scpool.tile([128, 1], dt.float32, tag="sc", name=f"rn_{i}_{m}")
                nc.vector.reciprocal(rnorm[:], nrme[:])
                return rnorm

            def delta_block(i):
                """dWk matmul + delta elementwise for aug i on all 4 chunks."""
                augt = [
                    wpool.tile([128, D], dt.bfloat16, tag="aug", name=f"aug_{i}_{k}")
                    for k in range(K8)
                ]
                for k in range(K8):
                    nc.sync.dma_start(augt[k][:], aug_t[i][ts(k, 128), :])
                outs = []
                for m in range(M4):
                    ps = pp_proj.tile([128, D], dt.float32, tag="ps_proj")
                    for k in range(K8):
                        for s0 in (0, 512):
                            nc.tensor.matmul(
                                ps[:, s0:s0 + 512], xT[k][:, ts(m, 128)],
                                augt[k][:, s0:s0 + 512],
                                start=(k == 0), stop=(k == K8 - 1),
                            )
                    ex = exsb[m]
                    scr = scrpool.tile([128, D], dt.bfloat16, tag="scr", name=f"scr_{i}_{m}")
                    ss = scpool.tile([128, 1], dt.float32, tag="sc", name=f"ss_{i}_{m}")
                    nc.scalar.activation(scr[:], ps[:], AF.Square, accum_out=ss[:])
                    kx = scpool.tile([128, 1], dt.float32, tag="sc", name=f"kx_{i}_{m}")
                    scr2 = scrpool.tile([128, D], dt.bfloat16, tag="scr", name=f"scr2_{i}_{m}")
                    nc.vector.scalar_tensor_tensor(
                        scr2[:], ps[:], 1.0, x32[m][:], ALU.mult, ALU.mult,
                        accum_out=kx[:],
                    )
                    rnorm = rnorm_chain(i, m, ss)
                    rk, rr = scalar_chain(i, m, ex[:, EX_DBW[i]:EX_DBW[i] + 1], kx, rnorm)
                    v = scpool.tile([128, 1], dt.float32, tag="sc", name=f"v_{i}_{m}")
                    nc.vector.tensor_scalar_add(
                        v[:], ex[:, EX_VW[i]:EX_VW[i] + 1], cvec[:, 4 + i:5 + i]
                    )
                    dv = scpool.tile([128, 1], dt.float32, tag="sc", name=f"dv_{i}_{m}")
                    nc.vector.tensor_tensor(dv[:], v[:], rk[:], ALU.subtract)
                    s = scpool.tile([128, 1], dt.float32, tag="sc", name=f"s_{i}_{m}")
                    nc.vector.tensor_tensor(s[:], dv[:], rr[:], ALU.mult)
                    if i in (0, 1):
                        o = qkpool.tile([128, D], dt.bfloat16, tag="qk", name=f"qk_{i}_{m}")
                        nc.vector.scalar_tensor_tensor(
                            o[:], ps[:], s[:], x32[m][:], ALU.mult, ALU.add
                        )
                        outs.append(o)
                    else:
                        # i == 2: u = A + s*B  (A/B live in the extras tile)
                        nc.vector.scalar_tensor_tensor(
                            u_bf[m][:], ex[:, EX_B:EX_B + H], s[:], ex[:, EX_A:EX_A + H],
                            ALU.mult, ALU.add,
                        )
                qk_out[i] = outs

            def delta3_chunk(m):
                """dWk3 matmul; elementwise on DVE from SBUF copy; LN moments."""
                psd = pp_proj.tile([128, D], dt.float32, tag="ps_proj")
                for k in range(K8):
                    for s0 in (0, 512):
                        nc.tensor.matmul(
                            psd[:, s0:s0 + 512], xT[k][:, ts(m, 128)],
                            aug3t[k][:, s0:s0 + 512],
                            start=(k == 0), stop=(k == K8 - 1),
                        )
                mka = scpool.tile([128, 1], dt.float32, tag="sc", name=f"mka_{m}")
                mkb = scpool.tile([128, 1], dt.float32, tag="sc", name=f"mkb_{m}")
                nc.vector.tensor_scalar(
                    k3raw[m][:, 0:512], psd[:, 0:512], 1.0, 0.0, ALU.mult,
                    ALU.add, accum_out=mka[:],
                )
                nc.vector.tensor_scalar(
                    k3raw[m][:, 512:1024], psd[:, 512:1024], 1.0, 0.0, ALU.mult,
                    ALU.add, accum_out=mkb[:],
                )
                nc.vector.tensor_tensor(mks3[m][:], mka[:], mkb[:], ALU.add)
                scr = scrpool.tile([128, D], dt.bfloat16, tag="scr", name=f"sc3r_{m}")
                nc.vector.scalar_tensor_tensor(
                    scr[:], k3raw[m][:], 1.0, k3raw[m][:], ALU.mult, ALU.mult,
                    accum_out=kks3[m][:],
                )
                scr2 = scrpool.tile([128, D], dt.bfloat16, tag="scr", name=f"sc3r2_{m}")
                nc.vector.scalar_tensor_tensor(
                    scr2[:], k3raw[m][:], 1.0, xbf[m][:], ALU.mult, ALU.mult,
                    accum_out=xks3[m][:],
                )
                rnorm = rnorm_chain(3, m, kks3[m])
                rk, rr = scalar_chain(3, m, exsb[m][:, EX_DBW[3]:EX_DBW[3] + 1], xks3[m], rnorm)
                nc.vector.tensor_copy(a3s[m][:], rr[:])
                nc.vector.tensor_tensor(b3s[m][:], rr[:], rk[:], ALU.mult)

            def transpose_to(src_tiles, dst_tiles):
                for k in range(K8):
                    pst = pp_t.tile([128, TOK], dt.bfloat16, tag="pst")
                    for m in range(M4):
                        nc.tensor.transpose(
                            pst[:, ts(m, 128)], src_tiles[m][:, ts(k, 128)], ident_bf[:]
                        )
                    nc.vector.tensor_copy(dst_tiles[k][:], pst[:])

            # ---- k path first so the AllGather starts early
            delta_block(1)
            ktloc = [ktlpool.tile([128, TOK], SC_DT, tag="ktloc", name=f"ktloc_{k}") for k in range(K8)]
            transpose_to(qk_out[1], ktloc)
            for k in range(K8):
                nc.sync.dma_start(agk_in[ts(k, 128), :], ktloc[k][:])
            nc.gpsimd.collective_compute(
                "AllGather", ALU.bypass, ins=[agk_in[0:256, :]], outs=[agk_pc[0][:]],
                replica_groups=RG,
            )
            delta_block(2)
            for m in range(M4):
                nc.sync.dma_start(agu_in[ts(m, 128), :], u_bf[m][:])
            nc.gpsimd.collective_compute(
                "AllGather", ALU.bypass, ins=[agu_in[:]], outs=[agu_out[:]],
                replica_groups=RG,
            )
            for j in range(1, 4):
                nc.gpsimd.collective_compute(
                    "AllGather", ALU.bypass,
                    ins=[agk_in[256 * j:256 * (j + 1), :]], outs=[agk_pc[j][:]],
                    replica_groups=RG,
                )
            delta_block(0)
            transpose_to(qk_out[0], qT)
            for m in range(M4):
                delta3_chunk(m)
                xsq = scrpool.tile([128, D], dt.bfloat16, tag="scr", name=f"xsq_{m}")
                nc.scalar.activation(xsq[:], x32[m][:], AF.Square, accum_out=xxs[m][:])

        # ---------------- attention ----------------
        with (
            tc.tile_pool(name="attn_sb", bufs=1) as attn_sb,
            tc.tile_pool(name="epool", bufs=4) as epool,
            tc.tile_pool(name="fin", bufs=2) as fin,
            tc.tile_pool(name="pp_sc", bufs=3, space="PSUM") as pp_sc,
            tc.tile_pool(name="pp_nr", bufs=2, space="PSUM") as pp_nr,
        ):
            kT = [attn_sb.tile([128, S], SC_DT, tag=f"kT_{k}", name=f"kTsb_{k}") for k in range(K8)]
            for k in range(K8):
                src = agk_pc[k // 2][:].rearrange("(c d) t -> d c t", c=4)[ts(k % 2, 128), :, :]
                dst = kT[k][:].rearrange("p (c t) -> p c t", c=4)
                nc.sync.dma_start(dst, src)
            uext = attn_sb.tile([128, NKC, H, 2], dt.bfloat16, tag="uext")
            nc.vector.memset(uext[:], 1.0)
            u_all = attn_sb.tile([128, NKC, H], dt.bfloat16, tag="u_all")
            nc.sync.dma_start(
                u_all[:], agu_out[:].rearrange("(kc p) h -> p kc h", p=128)
            )
            nc.vector.tensor_copy(uext[:, :, :, 0], u_all[:])

            SCALE = float(HD) ** -0.5

            for hp in range(K8):         # 8 head pairs; pair hp = heads 2hp, 2hp+1
                nr_ps = pp_nr.tile([128, TOK], dt.float32, tag="nr")
                hA, hB = 2 * hp, 2 * hp + 1
                for kc in range(NKC):
                    ps2 = pp_sc.tile([128, 2, TOK], dt.float32, tag="sc2")
                    nc.tensor.matmul(
                        ps2[:, 0, :], kT[hp][0:64, ts(kc, 128)], qT[hp][0:64, :],
                        start=True, stop=True, tile_position=(0, 0),
                    )
                    nc.tensor.matmul(
                        ps2[:, 1, :], kT[hp][64:128, ts(kc, 128)], qT[hp][64:128, :],
                        start=True, stop=True, tile_position=(64, 0),
                    )
                    E = epool.tile([128, 2, TOK], dt.bfloat16, tag="E")
                    nc.scalar.activation(E[:], ps2[:], AF.Exp, scale=SCALE)
                    nc.tensor.matmul(
                        nr_ps[0:2, :], uext[:, kc, hA, :], E[:, 0, :],
                        start=(kc == 0), stop=(kc == NKC - 1),
                        tile_position=(0, 0),
                    )
                    nc.tensor.matmul(
                        nr_ps[32:34, :], uext[:, kc, hB, :], E[:, 1, :],
                        start=(kc == 0), stop=(kc == NKC - 1),
                        tile_position=(0, 32),
                    )
                # stage the pair's n/r rows and fold into v3acc (overlaps attention)
                for j in range(2):
                    nc.vector.tensor_copy(
                        nrw[0:2, j * TOK:(j + 1) * TOK], nr_ps[32 * j:32 * j + 2, :]
                    )
                for m in range(M4):
                    psT = pp_nr.tile([128, TOK], dt.float32, tag="nr")
                    for j in range(2):
                        nc.tensor.transpose(
                            psT[:, 2 * j:2 * j + 2],
                            nrw[0:2, j * TOK + 128 * m: j * TOK + 128 * (m + 1)],
                            ident_f32[0:2, 0:2],
                        )
                    nrT = fin.tile([128, 4], dt.float32, tag="nrT", name=f"nrT_{hp}_{m}")
                    nc.vector.tensor_copy(nrT[:], psT[:, 0:4])
                    rec = fin.tile([128, 2], dt.float32, tag="rec", name=f"rec_{hp}_{m}")
                    nc.vector.reciprocal(rec[:], nrT[:, 1:4:2])
                    prod = fin.tile([128, 2], dt.float32, tag="prod", name=f"pr_{hp}_{m}")
                    nc.vector.tensor_tensor(prod[:], nrT[:, 0:4:2], rec[:], ALU.mult)
                    pv = fin.tile([128, 1], dt.float32, tag="pv", name=f"pv_{hp}_{m}")
                    nc.vector.tensor_reduce(pv[:], prod[:], axis=mybir.AxisListType.X, op=ALU.add)
                    nc.vector.tensor_tensor(v3acc[m][:], v3acc[m][:], pv[:], ALU.add)

            # ---- final delta + layernorm (stats from precomputed moments)
            for m in range(M4):
                v3 = fin.tile([128, 1], dt.float32, tag="v3", name=f"v3_{m}")
                nc.vector.tensor_scalar_add(v3[:], v3acc[m][:], cvec[:, 7:8])
                s3 = fin.tile([128, 1], dt.float32, tag="s3", name=f"s3_{m}")
                nc.vector.tensor_scalar_mul(s3[:], v3[:], a3s[m][:])
                nc.vector.tensor_tensor(s3[:], s3[:], b3s[m][:], ALU.subtract)
                # mu = (sum_x + s3*sum_k)/D
                mu = fin.tile([128, 1], dt.float32, tag="mu", name=f"mu_{m}")
                nc.vector.tensor_scalar_mul(mu[:], s3[:], mks3[m][:])
                nc.vector.tensor_tensor(mu[:], mu[:], mxs[m][:], ALU.add)
                nc.vector.tensor_scalar_mul(mu[:], mu[:], 1.0 / D)
                # E[y^2] = (xx + 2 s3 xk + s3^2 kk)/D ; var = E[y^2] - mu^2
                t1 = fin.tile([128, 1], dt.float32, tag="t1", name=f"t1_{m}")
                nc.vector.tensor_scalar_mul(t1[:], s3[:], kks3[m][:])
                t2 = fin.tile([128, 1], dt.float32, tag="t2", name=f"t2_{m}")
                nc.vector.tensor_scalar(t2[:], xks3[m][:], 2.0, None, ALU.mult)
                nc.vector.tensor_tensor(t2[:], t2[:], t1[:], ALU.add)
                nc.vector.tensor_scalar_mul(t2[:], t2[:], s3[:])
                nc.vector.tensor_tensor(t2[:], t2[:], xxs[m][:], ALU.add)
                var = fin.tile([128, 1], dt.float32, tag="var", name=f"var_{m}")
                nc.vector.tensor_scalar_mul(var[:], t2[:], 1.0 / D)
                mu2 = fin.tile([128, 1], dt.float32, tag="mu2", name=f"mu2_{m}")
                nc.vector.tensor_tensor(mu2[:], mu[:], mu[:], ALU.mult)
                nc.vector.tensor_tensor(var[:], var[:], mu2[:], ALU.subtract)
                nc.vector.tensor_scalar_add(var[:], var[:], LN_EPS)
                lnv2 = fin.tile([128, 1], dt.float32, tag="lnv2", name=f"lnv2_{m}")
                nc.scalar.activation(lnv2[:], var[:], AF.Ln)
                rstd = fin.tile([128, 1], dt.float32, tag="rstd", name=f"rstd_{m}")
                nc.scalar.activation(rstd[:], lnv2[:], AF.Exp, scale=-0.5)
                s3r = fin.tile([128, 1], dt.float32, tag="s3r", name=f"s3r_{m}")
                nc.vector.tensor_scalar_mul(s3r[:], s3[:], rstd[:])
                # yn = (x - mu)*rstd + k3raw*(s3*rstd) ; out = yn*g + b
                yn = fin.tile([128, D], dt.float32, tag="yn", name=f"yn_{m}")
                nc.vector.tensor_scalar(yn[:], x32[m][:], mu[:], rstd[:], ALU.subtract, ALU.mult)
                nc.vector.scalar_tensor_tensor(
                    yn[:], k3raw[m][:], s3r[:], yn[:], ALU.mult, ALU.add
                )
                yg = fin.tile([128, D], dt.float32, tag="yg", name=f"yg_{m}")
                nc.vector.tensor_tensor(yg[:], yn[:], lng[:], ALU.mult)
                nc.vector.tensor_tensor(yg[:], yg[:], lnb[:], ALU.add)
                nc.sync.dma_start(y_t[ts(m, 128), :], yg[:])

    _split_multi_waits(nc)
    nc.finalize()
    return nc


def _host_prep(inputs):
    """Precompute augmented weights and constants; returns per-core in_maps."""
    f32 = np.float32
    x = np.asarray(inputs["x"], f32)
    Wq, bq = np.asarray(inputs["Wq"], f32), np.asarray(inputs["bq"], f32)
    Wk, bk = np.asarray(inputs["Wk"], f32), np.asarray(inputs["bk"], f32)
    Wv, bv = np.asarray(inputs["Wv"], f32), np.asarray(inputs["bv"], f32)
    Wo, bo = np.asarray(inputs["Wo"], f32), np.asarray(inputs["bo"], f32)
    dWk, dbw = np.asarray(inputs["dWk"], f32), np.asarray(inputs["dbw"], f32)
    dbb, dWv = np.asarray(inputs["dbb"], f32), np.asarray(inputs["dWv"], f32)
    dbv = np.asarray(inputs["dbv"], f32)
    ln_g, ln_b = np.asarray(inputs["ln_g"], f32), np.asarray(inputs["ln_b"], f32)

    w = Wo @ dWv[3]                                   # (D,)
    Wu = np.zeros((D, H), f32)
    for h in range(H):
        Wu[h * HD:(h + 1) * HD, h] = w[h * HD:(h + 1) * HD]
    Bu = dWk[2] @ Wu                                  # (D, H)

    vw = [Wq @ dWv[0], Wk @ dWv[1], Wv @ dWv[2]]
    vc = [float(bq @ dWv[0] + dbv[0]), float(bk @ dWv[1] + dbv[1]),
          float(bv @ dWv[2] + dbv[2])]
    c3 = float(bo @ dWv[3] + dbv[3])

    bf = ml_dtypes.bfloat16
    augs = [np.ascontiguousarray(dWk[i]).astype(bf) for i in range(4)]
    ex = np.zeros((D, W_EX), f32)
    for i in range(4):
        ex[:, EX_DBW[i]] = dbw[i]
    for i in range(3):
        ex[:, EX_VW[i]] = vw[i]
    ex[:, EX_A:EX_A + H] = Wu
    ex[:, EX_B:EX_B + H] = Bu
    ex = ex.astype(bf)

    cvec = np.zeros((128, 16), f32)
    for i in range(4):
        cvec[:, i] = -dbb[i]
    for i in range(3):
        cvec[:, 4 + i] = vc[i]
    cvec[:, 7] = c3

    lng = np.broadcast_to(ln_g[None, :], (128, D)).copy()
    lnb = np.broadcast_to(ln_b[None, :], (128, D)).copy()

    xf = x.reshape(B * S, D)
    in_maps = []
    for c in range(N_CORES):
        m = {
            "x": np.ascontiguousarray(xf[c * TOK:(c + 1) * TOK]),
            "ex": ex, "cvec": cvec, "lng": lng, "lnb": lnb,
        }
        for i in range(4):
            m[f"aug{i}"] = augs[i]
        in_maps.append(m)
    return in_maps


def kernel(**inputs):
    global LAST_RESULTS
    if "nc" not in _CACHE:
        _CACHE["nc"] = _build_program()
    nc = _CACHE["nc"]
    in_maps = _host_prep(inputs)
    res = run_bass_kernel_spmd(nc, in_maps, core_ids=list(range(N_CORES)))
    LAST_RESULTS = res
    out = np.concatenate(
        [res.results[c]["y"] for c in range(N_CORES)], axis=0
    ).reshape(B, S, D)
    return out.astype(np.float32)
